# revision 34
# baseline (speedup 1.0000x reference)
"""Trainium2 Bass kernel for a pre-LN transformer block (nn_BaseBlock).

Reference computation (per batch b, fp32):
    h   = LN1(x); k,q,v = h@Wk+bk, h@Wq+bq, h@Wv+bv
    sim = (k @ q^T)/sqrt(E)  (causal tril mask), att = softmax(sim) @ v
    x2  = x + att
    h2  = LN2(x2)
    f   = gelu(gelu(gelu(h2@W1+b1)@W2a+b2a)@W2b+b2b)@W3 + b3
    out = x2 + f

Sharding over 8 cores: core c handles batch b=c//2, row half r=c%2
(i-tiles {2*it + r} of that batch, so the padded causal extent profile is
core-independent).  All matmuls run in fp8e4 with DoubleRow perf mode
(two 128-deep k-tiles contracted per pass); PSUM accumulation is fp32.
Weights are host-scaled by per-tensor powers of two into fp8's sweet spot;
the descale factors ride in a small `scales` input consumed as per-partition
ACT/DVE scale operands, so the compiled program is weight-independent.

LN1 runs feature-major: the host supplies x transposed (xT), token-major
stats are computed on the ACT engine, routed through a DRAM roundtrip into
row-broadcast tiles, and the normalize is two DVE passes straight into the
fp8 feature-major activation tile — no PE transposes for LN1.  Softmax-P
and LN2(h2) still transpose on the PE (bf16, PSUM bounce).

The causal mask enters only through a tiny per-core diagonal-block input
(mask_d, [RT,128,256]); fully-open score blocks skip masking entirely and
exp() reads PSUM directly.  The residual stream x2 stays resident in SBUF
in fp32 (no DRAM spill).
"""

import time

import numpy as np
import ml_dtypes

import concourse.bass as bass
import concourse.mybir as mybir
from concourse import bacc
import concourse.tile as tile
from concourse.bass_utils import run_bass_kernel_spmd

F32 = mybir.dt.float32
BF16 = mybir.dt.bfloat16
F8 = mybir.dt.float8e4
F8E5 = mybir.dt.float8e5
AF = mybir.ActivationFunctionType
ALU = mybir.AluOpType
AX = mybir.AxisListType
DR = mybir.MatmulPerfMode.DoubleRow

EPS = 1e-5
N_CORES = 8


class Cfg:
    def __init__(self, E=1024, H=4096, T=2048, R=1024):
        self.E, self.H, self.T, self.R = E, H, T, R
        self.ET, self.HT, self.CT, self.RT = E // 128, H // 128, T // 128, R // 128
        self.scale = 1.0 / np.sqrt(E)


def _blocks(total, bs=512):
    return [(o, min(bs, total - o)) for o in range(0, total, bs)]


def build_program(cfg: Cfg, reps: int = 1):
    """Build the SPMD Bass program (one core's view).

    reps>1 wraps the body in unrolled copies — used only for timing
    (amortizes the ~80ms axon dispatch round-trip over reps executions).
    """
    E, H, T, R = cfg.E, cfg.H, cfg.T, cfg.R
    ET, HT, CT, RT = cfg.ET, cfg.HT, cfg.CT, cfg.RT

    nc = bacc.Bacc("TRN2", target_bir_lowering=False, debug=False,
                   num_devices=N_CORES)

    # ---- DRAM I/O ----
    xT8_in = nc.dram_tensor("xT8_in", [ET, 128, T], F8, kind="ExternalInput")
    xo8_in = nc.dram_tensor("xo8_in", [ET, 128, R], F8, kind="ExternalInput")
    x_b = nc.dram_tensor("x_b", [T, E], F32, kind="ExternalInput")
    x_own = nc.dram_tensor("x_own", [R, E], F32, kind="ExternalInput")
    mask_d = nc.dram_tensor("mask_d", [RT, 128, 256], BF16, kind="ExternalInput")
    wqt = nc.dram_tensor("wqt", [ET, 128, ET, 128], F8, kind="ExternalInput")
    wkt = nc.dram_tensor("wkt", [ET, 128, ET, 128], F8, kind="ExternalInput")
    wv = nc.dram_tensor("wv", [ET, 128, E], F8, kind="ExternalInput")
    bq = nc.dram_tensor("bq", [E], F32, kind="ExternalInput")
    bk = nc.dram_tensor("bk", [E], F32, kind="ExternalInput")
    bv = nc.dram_tensor("bv", [E], F32, kind="ExternalInput")
    w1t = nc.dram_tensor("w1t", [HT, 128, ET, 128], F8, kind="ExternalInput")
    w2at = nc.dram_tensor("w2at", [HT, 128, HT, 128], F8, kind="ExternalInput")
    w2bt = nc.dram_tensor("w2bt", [HT, 128, HT, 128], F8, kind="ExternalInput")
    w3m = nc.dram_tensor("w3m", [HT, 128, E], F8, kind="ExternalInput")
    b1 = nc.dram_tensor("b1", [H], F32, kind="ExternalInput")
    b2a = nc.dram_tensor("b2a", [H], F32, kind="ExternalInput")
    b2b = nc.dram_tensor("b2b", [H], F32, kind="ExternalInput")
    b3 = nc.dram_tensor("b3", [E], F32, kind="ExternalInput")
    scales = nc.dram_tensor("scales", [8], F32, kind="ExternalInput")
    ident_in = nc.dram_tensor("ident_in", [128, 128], BF16, kind="ExternalInput")
    nck = nc.dram_tensor("nck", [E], F32, kind="ExternalInput")
    ncq = nc.dram_tensor("ncq", [E], F32, kind="ExternalInput")
    ncv = nc.dram_tensor("ncv", [E], F32, kind="ExternalInput")
    nc1 = nc.dram_tensor("nc1", [H], F32, kind="ExternalInput")
    stat_ri = nc.dram_tensor("stat_ri", [T], F32, kind="Internal")
    stat_r2 = nc.dram_tensor("stat_r2", [T], F32, kind="Internal")
    stat_rio = nc.dram_tensor("stat_rio", [R], F32, kind="Internal")
    stat_r2o = nc.dram_tensor("stat_r2o", [R], F32, kind="Internal")
    stat_ri2 = nc.dram_tensor("stat_ri2", [R], F32, kind="Internal")
    stat_r22 = nc.dram_tensor("stat_r22", [R], F32, kind="Internal")
    out = nc.dram_tensor("out", [R, E], F32, kind="ExternalOutput")

    d = locals()
    with tile.TileContext(nc) as tc:
        for _ in range(reps):
            _emit(tc, cfg, d)
    nc.compile()
    return nc


def _emit(tc, cfg, d):
    nc = tc.nc
    E, H, T, R = cfg.E, cfg.H, cfg.T, cfg.R
    ET, HT, CT, RT = cfg.ET, cfg.HT, cfg.CT, cfg.RT
    c32 = float(cfg.scale)
    KP_E = ET // 2   # DoubleRow k-pair count for E contraction
    KP_H = HT // 2

    import contextlib
    ctx = contextlib.ExitStack()
    with ctx:
        consts = ctx.enter_context(tc.tile_pool(name="consts", bufs=1))
        mm_ps = ctx.enter_context(tc.tile_pool(name="mm_ps", bufs=6, space="PSUM"))
        tr_ps = ctx.enter_context(tc.tile_pool(name="tr_ps", bufs=2, space="PSUM"))
        stp = ctx.enter_context(tc.tile_pool(name="ln_stats", bufs=4))
        fxp = ctx.enter_context(tc.tile_pool(name="fixup", bufs=2))

        eps_t = consts.tile([128, 1], F32)
        nc.vector.memset(eps_t[:], EPS)
        ident = consts.tile([128, 128], BF16)
        nc.sync.dma_start(out=ident[:], in_=d["ident_in"].ap())
        neg1 = consts.tile([128, 1], F32)
        nc.vector.memset(neg1[:], -1.0)

        def bcast(name, dr, width=None):
            w = width or dr.shape[0]
            t = consts.tile([128, w], F32, tag=name)
            src = dr.ap()
            src_b = bass.AP(tensor=src.tensor, offset=src.offset,
                            ap=[[0, 128]] + list(src.ap))
            nc.sync.dma_start(out=t[:], in_=src_b)
            return t

        def cols(name, dr, nt):
            t = consts.tile([128, nt], F32, tag=name)
            nc.sync.dma_start(out=t[:], in_=dr.ap().rearrange("(t p) -> p t", p=128))
            return t

        sc = bcast("sc", d["scales"], width=8)
        SQ, SK, SV = sc[:, 0:1], sc[:, 1:2], sc[:, 2:3]
        S1, S2A, S2B, S3 = sc[:, 3:4], sc[:, 4:5], sc[:, 5:6], sc[:, 6:7]
        bq_c = cols("bq", d["bq"], ET)
        bk_c = cols("bk", d["bk"], ET)
        bv_bc = bcast("bv", d["bv"])
        b3_bc = bcast("b3", d["b3"])
        b1_c = cols("b1", d["b1"], HT)
        b2a_c = cols("b2a", d["b2a"], HT)
        b2b_c = cols("b2b", d["b2b"], HT)
        nck_c = cols("nck", d["nck"], ET)
        ncq_c = cols("ncq", d["ncq"], ET)
        ncv_bc = bcast("ncv", d["ncv"])
        nc1_c = cols("nc1", d["nc1"], HT)

        def tile_stats(src_ap, ri_slot, r2_slot):
            """Token-major LN stats of one [128, E] tile.

            ri = 1/sqrt(var+eps), r2 = mu*ri — the two per-token factors the
            folded-LN fixup needs (LN itself never materializes on-chip)."""
            scr = stp.tile([128, E], BF16, tag="scr")
            s1 = stp.tile([128, 1], F32, tag="s1")
            nc.scalar.activation(out=scr[:], in_=src_ap, func=AF.Copy, bias=0.0,
                                 scale=1.0, accum_out=s1[:])
            s2 = stp.tile([128, 1], F32, tag="s2")
            nc.scalar.activation(out=scr[:], in_=src_ap, func=AF.Square,
                                 accum_out=s2[:])
            mu = stp.tile([128, 1], F32, tag="mu_s")
            nc.scalar.mul(out=mu[:], in_=s1[:], mul=1.0 / E)
            mu2 = stp.tile([128, 1], F32, tag="mu2")
            nc.vector.tensor_mul(out=mu2[:], in0=mu[:], in1=mu[:])
            var = stp.tile([128, 1], F32, tag="var")
            nc.vector.scalar_tensor_tensor(out=var[:], in0=s2[:], scalar=1.0 / E,
                                           in1=mu2[:], op0=ALU.mult,
                                           op1=ALU.subtract)
            sd = stp.tile([128, 1], F32, tag="sd")
            nc.scalar.activation(out=sd[:], in_=var[:], func=AF.Sqrt,
                                 bias=eps_t[:], scale=1.0)
            nc.vector.reciprocal(out=ri_slot, in_=sd[:])
            nc.vector.tensor_mul(out=r2_slot, in0=mu[:], in1=ri_slot)

        x2_pool = ctx.enter_context(tc.tile_pool(name="x2", bufs=1))
        x2 = x2_pool.tile([128, RT, E], F32)  # residual stream (own rows)
        h2T_pool = ctx.enter_context(tc.tile_pool(name="h2T_pool", bufs=1))
        h2T = h2T_pool.tile([128, ET, R], F8, tag="h2T")
        ri2_b = h2T_pool.tile([128, R], F32, tag="ri2_b")
        r2_2b = h2T_pool.tile([128, R], F32, tag="r2_2b")

        # ================= attention block =================
        with tc.tile_pool(name="attn_big", bufs=1) as abig:
            xT8 = abig.tile([128, ET, T], F8, tag="xT8")
            xo8 = abig.tile([128, ET, R], F8, tag="xo8")

            qkvp = tc.tile_pool(name="qkvp", bufs=1)
            qkv_pool = qkvp.__enter__()
            statp = tc.tile_pool(name="statp", bufs=1)
            stat_pool = statp.__enter__()

            # x arrives pre-quantized to fp8 from the host (layout prep):
            # 3 MB across both HWDGE queues unblocks the projections in ~5 us
            for kt in range(ET):
                eng = nc.sync if kt % 2 == 0 else nc.scalar
                eng.dma_start(out=xT8[:, kt, :], in_=d["xT8_in"].ap()[kt])

            rio_all = stat_pool.tile([128, RT], F32, tag="rio_all")
            r2o_all = stat_pool.tile([128, RT], F32, tag="r2o_all")
            ri_all = stat_pool.tile([128, CT], F32, tag="ri_all")
            r2_all = stat_pool.tile([128, CT], F32, tag="r2_all")

            def brow(name, dr, w):
                t = stat_pool.tile([128, w], F32, tag=name, name="brow_t")
                src = dr.ap()
                src_b = bass.AP(tensor=src.tensor, offset=src.offset,
                                ap=[[0, 128]] + list(src.ap))
                nc.sync.dma_start(out=t[:], in_=src_b)
                return t

            qT8 = qkv_pool.tile([128, ET, T], F8, tag="qT8")
            kT8o = qkv_pool.tile([128, ET, R], F8, tag="kT8o")
            vtm = qkv_pool.tile([128, CT, E], F8, tag="vtm")  # token-major v

            def fixup_fm(ps, dst, ri_row, r2_row, ncol, bcol, S, func):
                """Feature-major eviction with folded LN:
                dst = func(S*ps*ri − c_m*r2 + b_m)   (exact LN algebra)."""
                o1 = fxp.tile([128, 512], F32, tag="fx1", name="fx_t")
                nc.vector.scalar_tensor_tensor(out=o1[:], in0=ps, scalar=S,
                                               in1=ri_row, op0=ALU.mult,
                                               op1=ALU.mult)
                o2 = fxp.tile([128, 512], F32, tag="fx2", name="fx_t")
                nc.vector.scalar_tensor_tensor(out=o2[:], in0=r2_row,
                                               scalar=ncol, in1=o1[:],
                                               op0=ALU.mult, op1=ALU.add)
                nc.scalar.activation(out=dst, in_=o2[:], func=func,
                                     bias=bcol, scale=1.0)

            # ---- v (full ctx, token-major), LN stats pipelined per tile ----
            with tc.tile_pool(name="wv_pool", bufs=1) as wvp, \
                 tc.tile_pool(name="ln1w", bufs=3) as lnw:
                wv_sb = wvp.tile([128, ET, E], F8)
                for kt in range(ET):
                    nc.scalar.dma_start(out=wv_sb[:, kt, :], in_=d["wv"].ap()[kt])
                for tt in range(CT):
                    xt = lnw.tile([128, E], F32, tag="xt", bufs=2)
                    nc.gpsimd.dma_start(out=xt[:],
                                        in_=d["x_b"].ap()[tt * 128:(tt + 1) * 128, :])
                    tile_stats(xt[:], ri_all[:, tt:tt + 1], r2_all[:, tt:tt + 1])
                    pss = [mm_ps.tile([128, 512], F32, tag="mm", name="mm_ps_t")
                           for _ in range(2)]
                    for kp in range(KP_E):
                        for bi, eo in enumerate((0, 512)):
                            nc.tensor.matmul(
                                pss[bi][:, :],
                                xT8[:, 2 * kp:2 * kp + 2, tt * 128:(tt + 1) * 128],
                                wv_sb[:, 2 * kp:2 * kp + 2, eo:eo + 512],
                                start=(kp == 0), stop=(kp == KP_E - 1),
                                perf_mode=DR)
                    for bi, eo in enumerate((0, 512)):
                        o1 = fxp.tile([128, 512], F32, tag="fx1", name="fx_t")
                        nc.vector.tensor_scalar(out=o1[:], in0=pss[bi][:, :],
                                                scalar1=SV,
                                                scalar2=ri_all[:, tt:tt + 1],
                                                op0=ALU.mult, op1=ALU.mult)
                        o2 = fxp.tile([128, 512], F32, tag="fx2", name="fx_t")
                        nc.vector.scalar_tensor_tensor(
                            out=o2[:], in0=ncv_bc[:, eo:eo + 512],
                            scalar=r2_all[:, tt:tt + 1], in1=o1[:],
                            op0=ALU.mult, op1=ALU.add)
                        nc.vector.tensor_tensor(out=vtm[:, tt, eo:eo + 512],
                                                in0=o2[:],
                                                in1=bv_bc[:, eo:eo + 512],
                                                op=ALU.add)
                nc.sync.dma_start(
                    out=d["stat_ri"].ap().rearrange("(t p) -> p t", p=128),
                    in_=ri_all[:])
                nc.sync.dma_start(
                    out=d["stat_r2"].ap().rearrange("(t p) -> p t", p=128),
                    in_=r2_all[:])
                ri_b = brow("ri_b", d["stat_ri"], T)
                r2_b = brow("r2_b", d["stat_r2"], T)

                # ---- q (full ctx), feature-major ----
                with tc.tile_pool(name="wq_s", bufs=2) as wqs:
                    for mt in range(ET):
                        wq_mt = wqs.tile([128, ET, 128], F8, tag="wq_mt")
                        nc.scalar.dma_start(out=wq_mt[:], in_=d["wqt"].ap()[mt])
                        pss = [mm_ps.tile([128, 512], F32, tag="mm",
                                          name="mm_ps_t") for _ in range(4)]
                        for kp in range(KP_E):
                            for bi in range(4):
                                nc.tensor.matmul(
                                    pss[bi][:, :], wq_mt[:, 2 * kp:2 * kp + 2, :],
                                    xT8[:, 2 * kp:2 * kp + 2,
                                        bi * 512:(bi + 1) * 512],
                                    start=(kp == 0), stop=(kp == KP_E - 1),
                                    perf_mode=DR)
                        for bi in range(4):
                            fixup_fm(pss[bi][:, :],
                                     qT8[:, mt, bi * 512:(bi + 1) * 512],
                                     ri_b[:, bi * 512:(bi + 1) * 512],
                                     r2_b[:, bi * 512:(bi + 1) * 512],
                                     ncq_c[:, mt:mt + 1], bq_c[:, mt:mt + 1],
                                     SQ, AF.Identity)

                # own rows (k-path): pre-quantized input, stats, roundtrip
                for kt in range(ET):
                    eng = nc.sync if kt % 2 == 0 else nc.scalar
                    eng.dma_start(out=xo8[:, kt, :], in_=d["xo8_in"].ap()[kt])
                for it in range(RT):
                    xt = lnw.tile([128, E], F32, tag="xt", bufs=2)
                    nc.gpsimd.dma_start(out=xt[:],
                                        in_=d["x_own"].ap()[it * 128:(it + 1) * 128, :])
                    tile_stats(xt[:], rio_all[:, it:it + 1], r2o_all[:, it:it + 1])
                nc.sync.dma_start(
                    out=d["stat_rio"].ap().rearrange("(t p) -> p t", p=128),
                    in_=rio_all[:])
                nc.sync.dma_start(
                    out=d["stat_r2o"].ap().rearrange("(t p) -> p t", p=128),
                    in_=r2o_all[:])
                ri_bo = brow("ri_bo", d["stat_rio"], R)
                r2_bo = brow("r2_bo", d["stat_r2o"], R)

                # ---- k (own rows), feature-major ----
                with tc.tile_pool(name="wk_s", bufs=2) as wks:
                    for mt in range(ET):
                        wk_mt = wks.tile([128, ET, 128], F8, tag="wk_mt")
                        nc.scalar.dma_start(out=wk_mt[:], in_=d["wkt"].ap()[mt])
                        pss = [mm_ps.tile([128, 512], F32, tag="mm",
                                          name="mm_ps_t") for _ in range(2)]
                        for kp in range(KP_E):
                            for bi, ro in enumerate((0, 512)):
                                nc.tensor.matmul(
                                    pss[bi][:, :], wk_mt[:, 2 * kp:2 * kp + 2, :],
                                    xo8[:, 2 * kp:2 * kp + 2, ro:ro + 512],
                                    start=(kp == 0), stop=(kp == KP_E - 1),
                                    perf_mode=DR)
                        for bi, ro in enumerate((0, 512)):
                            fixup_fm(pss[bi][:, :], kT8o[:, mt, ro:ro + 512],
                                     ri_bo[:, ro:ro + 512], r2_bo[:, ro:ro + 512],
                                     nck_c[:, mt:mt + 1], bk_c[:, mt:mt + 1],
                                     SK, AF.Identity)

            statp.__exit__(None, None, None)

            # ---- attention rows (own i-tiles), software-pipelined ----
            # Core r owns batch i-tiles {2*it + r}; padded causal extent
            # ext(it) = 2*(it+1) j-tiles is core-independent.  Only the two
            # diagonal j-tiles need masking (mask_d input); earlier blocks are
            # fully open and exp() reads the score PSUM directly.  No
            # max-subtraction: |sim/32| <= ~11 keeps exp in fp32/bf16 range.
            # Scores of i-tile it+1 are emitted before AV of it so the PE works
            # while the pT XBAR-DMA transposes of it are in flight.
            with tc.tile_pool(name="at_p", bufs=2) as pp, \
                 tc.tile_pool(name="at_misc", bufs=3) as msc, \
                 tc.tile_pool(name="at_md", bufs=2) as mdp, \
                 tc.tile_pool(name="ln2", bufs=2) as l2p:
                ri2_all = l2p.tile([128, RT], F32, tag="ri2_all", bufs=1)
                r22_all = l2p.tile([128, RT], F32, tag="r22_all", bufs=1)
                s1_all = l2p.tile([128, RT], F32, tag="s1_all", bufs=1)
                s2_all = l2p.tile([128, RT], F32, tag="s2_all", bufs=1)
                def scores_phase(it):
                    ext = 2 * (it + 1)
                    ncols = ext * 128
                    blks = _blocks(ncols)
                    nblk = len(blks)
                    md = mdp.tile([128, 256], BF16, tag="md", name="at_t")
                    nc.sync.dma_start(out=md[:], in_=d["mask_d"].ap()[it])
                    pss = [mm_ps.tile([128, 512], F32, tag="mm", name="mm_ps_t")
                           for _ in range(nblk)]
                    for kp in range(KP_E):
                        for bi, (jo, jn) in enumerate(blks):
                            nc.tensor.matmul(
                                pss[bi][:, :jn],
                                kT8o[:, 2 * kp:2 * kp + 2, it * 128:(it + 1) * 128],
                                qT8[:, 2 * kp:2 * kp + 2, jo:jo + jn],
                                start=(kp == 0), stop=(kp == KP_E - 1),
                                perf_mode=DR)
                    pbf = pp.tile([128, T], BF16, tag="pbf", name="at_t")
                    lacc = msc.tile([128, 8], F32, tag="lacc", name="at_t")
                    ns = 0
                    for bi, (jo, jn) in enumerate(blks):
                        last = (bi == nblk - 1)
                        jn_open = jn - 256 if last else jn
                        if jn_open > 0:
                            nc.scalar.activation(
                                out=pbf[:, jo:jo + jn_open],
                                in_=pss[bi][:, :jn_open], func=AF.Exp,
                                scale=c32, bias=neg1[:],
                                accum_out=lacc[:, ns:ns + 1])
                            ns += 1
                        if last:
                            simd = msc.tile([128, 256], F32, tag="simd",
                                            name="at_t")
                            nc.vector.tensor_tensor(out=simd[:],
                                                    in0=pss[bi][:, jn_open:jn],
                                                    in1=md[:], op=ALU.add)
                            nc.scalar.activation(
                                out=pbf[:, ncols - 256:ncols], in_=simd[:],
                                func=AF.Exp, scale=c32, bias=neg1[:],
                                accum_out=lacc[:, ns:ns + 1])
                            ns += 1
                    lrow = msc.tile([128, 1], F32, tag="lrow", name="at_t")
                    nc.vector.tensor_reduce(out=lrow[:], in_=lacc[:, :ns],
                                            axis=AX.X, op=ALU.add)
                    linv = msc.tile([128, 1], F32, tag="linv", name="at_t")
                    nc.vector.reciprocal(out=linv[:], in_=lrow[:])
                    pT = pp.tile([128, T], F8E5, tag="pT", name="at_t")
                    for jt in range(ext):
                        tp = tr_ps.tile([128, 128], BF16, tag="tr", name="tr_t")
                        nc.tensor.transpose(tp[:], pbf[:, jt * 128:(jt + 1) * 128],
                                            ident[:])
                        nc.vector.tensor_copy(out=pT[:, jt * 128:(jt + 1) * 128],
                                              in_=tp[:])
                    return ext, pT, linv

                def av_phase(it, ext, pT, linv):
                    xo = msc.tile([128, E], F32, tag="xo", bufs=2, name="at_t")
                    nc.gpsimd.dma_start(out=xo[:],
                                        in_=d["x_own"].ap()[it * 128:(it + 1) * 128, :])
                    pse = [mm_ps.tile([128, 512], F32, tag="mm", name="mm_ps_t")
                           for _ in range(2)]
                    for jp in range(ext // 2):
                        pT_pair = pT[:, jp * 256:(jp + 1) * 256].rearrange(
                            "p (two c) -> p two c", two=2)
                        for bi, eo in enumerate((0, 512)):
                            nc.tensor.matmul(
                                pse[bi][:, :], pT_pair,
                                vtm[:, 2 * jp:2 * jp + 2, eo:eo + 512],
                                start=(jp == 0), stop=(jp == ext // 2 - 1),
                                perf_mode=DR)
                    for bi, eo in enumerate((0, 512)):
                        nc.vector.scalar_tensor_tensor(
                            out=x2[:, it, eo:eo + 512], in0=pse[bi][:, :],
                            scalar=linv[:], in1=xo[:, eo:eo + 512],
                            op0=ALU.mult, op1=ALU.add)
                    # LN2 for this row-tile, fused so DVE/ACT/XBAR do it while
                    # the PE continues with the next i-tile's scores
                    x2bf = l2p.tile([128, E], BF16, tag="x2bf", name="l2_t")
                    nc.vector.tensor_copy(out=x2bf[:], in_=x2[:, it, :])
                    h2bf = l2p.tile([128, E], BF16, tag="h2bf", name="l2_t")
                    for et in range(ET):
                        nc.sync.dma_start(out=h2bf[:, et * 128:(et + 1) * 128],
                                          in_=x2bf[:, et * 128:(et + 1) * 128],
                                          transpose=True)
                    nc.vector.tensor_copy(
                        out=h2T[:, :, it * 128:(it + 1) * 128],
                        in_=h2bf[:].rearrange("p (et c) -> p et c", c=128))
                    scr2 = stp.tile([128, E], BF16, tag="scr2", name="l2_t")
                    nc.scalar.activation(out=scr2[:], in_=x2[:, it, :],
                                         func=AF.Copy, bias=0.0, scale=1.0,
                                         accum_out=s1_all[:, it:it + 1])
                    scr3 = stp.tile([128, E], BF16, tag="scr3", name="l2_t")
                    nc.scalar.activation(out=scr3[:], in_=x2[:, it, :],
                                         func=AF.Square,
                                         accum_out=s2_all[:, it:it + 1])
                    nc.vector.tensor_tensor(out=x2[:, it, :], in0=x2[:, it, :],
                                            in1=b3_bc[:], op=ALU.add)

                pend = None
                for it in range(RT):
                    cur = scores_phase(it)
                    if pend is not None:
                        av_phase(pend[0], *pend[1])
                    pend = (it, cur)
                av_phase(pend[0], *pend[1])
                mu8 = stp.tile([128, RT], F32, tag="mu8", name="l2_t")
                nc.scalar.mul(out=mu8[:], in_=s1_all[:], mul=1.0 / E)
                mu28 = stp.tile([128, RT], F32, tag="mu28", name="l2_t")
                nc.vector.tensor_mul(out=mu28[:], in0=mu8[:], in1=mu8[:])
                var8 = stp.tile([128, RT], F32, tag="var8", name="l2_t")
                nc.vector.scalar_tensor_tensor(out=var8[:], in0=s2_all[:],
                                               scalar=1.0 / E, in1=mu28[:],
                                               op0=ALU.mult, op1=ALU.subtract)
                sd8 = stp.tile([128, RT], F32, tag="sd8", name="l2_t")
                nc.scalar.activation(out=sd8[:], in_=var8[:], func=AF.Sqrt,
                                     bias=eps_t[:], scale=1.0)
                nc.vector.reciprocal(out=ri2_all[:], in_=sd8[:])
                nc.vector.tensor_mul(out=r22_all[:], in0=mu8[:], in1=ri2_all[:])
                nc.sync.dma_start(
                    out=d["stat_ri2"].ap().rearrange("(t p) -> p t", p=128),
                    in_=ri2_all[:])
                nc.sync.dma_start(
                    out=d["stat_r22"].ap().rearrange("(t p) -> p t", p=128),
                    in_=r22_all[:])
                srcap = d["stat_ri2"].ap()
                nc.sync.dma_start(out=ri2_b[:], in_=bass.AP(
                    tensor=srcap.tensor, offset=srcap.offset,
                    ap=[[0, 128]] + list(srcap.ap)))
                srcap = d["stat_r22"].ap()
                nc.sync.dma_start(out=r2_2b[:], in_=bass.AP(
                    tensor=srcap.tensor, offset=srcap.offset,
                    ap=[[0, 128]] + list(srcap.ap)))
            qkvp.__exit__(None, None, None)

        # ================= MLP block =================
        with tc.tile_pool(name="gx", bufs=1) as gxp, \
             tc.tile_pool(name="mlp_ws", bufs=1) as ws:
            g1T = gxp.tile([128, HT, R], F8, tag="gx")
            for mt in range(HT):
                w1_mt = ws.tile([128, ET, 128], F8, tag="w1_mt", bufs=3)
                nc.scalar.dma_start(out=w1_mt[:], in_=d["w1t"].ap()[mt])
                pss = [mm_ps.tile([128, 512], F32, tag="mm", name="mm_ps_t")
                       for _ in range(2)]
                for kp in range(KP_E):
                    for bi, ro in enumerate((0, 512)):
                        nc.tensor.matmul(
                            pss[bi][:, :], w1_mt[:, 2 * kp:2 * kp + 2, :],
                            h2T[:, 2 * kp:2 * kp + 2, ro:ro + 512],
                            start=(kp == 0), stop=(kp == KP_E - 1), perf_mode=DR)
                for bi, ro in enumerate((0, 512)):
                    fixup_fm(pss[bi][:, :], g1T[:, mt, ro:ro + 512],
                             ri2_b[:, ro:ro + 512], r2_2b[:, ro:ro + 512],
                             nc1_c[:, mt:mt + 1], b1_c[:, mt:mt + 1],
                             S1, AF.Gelu)

            with tc.tile_pool(name="g2", bufs=1) as g2p:
                g2T = g2p.tile([128, HT, R], F8, tag="g2")
                for mt in range(HT):
                    w2_mt = ws.tile([128, HT, 128], F8, tag="w2a_mt", bufs=3)
                    nc.scalar.dma_start(out=w2_mt[:], in_=d["w2at"].ap()[mt])
                    pss = [mm_ps.tile([128, 512], F32, tag="mm", name="mm_ps_t")
                           for _ in range(2)]
                    for kp in range(KP_H):
                        for bi, ro in enumerate((0, 512)):
                            nc.tensor.matmul(
                                pss[bi][:, :], w2_mt[:, 2 * kp:2 * kp + 2, :],
                                g1T[:, 2 * kp:2 * kp + 2, ro:ro + 512],
                                start=(kp == 0), stop=(kp == KP_H - 1),
                                perf_mode=DR)
                    for bi, ro in enumerate((0, 512)):
                        nc.scalar.activation(out=g2T[:, mt, ro:ro + 512],
                                             in_=pss[bi][:, :], func=AF.Gelu,
                                             bias=b2a_c[:, mt:mt + 1], scale=S2A)

                g3T = gxp.tile([128, HT, R], F8, tag="gx")
                for mt in range(HT):
                    w2_mt = ws.tile([128, HT, 128], F8, tag="w2b_mt", bufs=3)
                    nc.scalar.dma_start(out=w2_mt[:], in_=d["w2bt"].ap()[mt])
                    pss = [mm_ps.tile([128, 512], F32, tag="mm", name="mm_ps_t")
                           for _ in range(2)]
                    for kp in range(KP_H):
                        for bi, ro in enumerate((0, 512)):
                            nc.tensor.matmul(
                                pss[bi][:, :], w2_mt[:, 2 * kp:2 * kp + 2, :],
                                g2T[:, 2 * kp:2 * kp + 2, ro:ro + 512],
                                start=(kp == 0), stop=(kp == KP_H - 1),
                                perf_mode=DR)
                    for bi, ro in enumerate((0, 512)):
                        nc.scalar.activation(out=g3T[:, mt, ro:ro + 512],
                                             in_=pss[bi][:, :], func=AF.Gelu,
                                             bias=b2b_c[:, mt:mt + 1], scale=S2B)

            # ---- f = g3 @ W3 (+b3 already in x2); out = x2 + f ----
            with tc.tile_pool(name="w3p", bufs=1) as w3p, \
                 tc.tile_pool(name="outp", bufs=2) as op:
                w3_sb = w3p.tile([128, HT, E], F8)
                for kt in range(HT):
                    nc.scalar.dma_start(out=w3_sb[:, kt, :], in_=d["w3m"].ap()[kt])
                for tt in range(RT):
                    pse = [mm_ps.tile([128, 512], F32, tag="mm", name="mm_ps_t")
                           for _ in range(2)]
                    for kp in range(KP_H):
                        for bi, eo in enumerate((0, 512)):
                            nc.tensor.matmul(
                                pse[bi][:, :],
                                g3T[:, 2 * kp:2 * kp + 2, tt * 128:(tt + 1) * 128],
                                w3_sb[:, 2 * kp:2 * kp + 2, eo:eo + 512],
                                start=(kp == 0), stop=(kp == KP_H - 1),
                                perf_mode=DR)
                    for bi, eo in enumerate((0, 512)):
                        ot = op.tile([128, 512], F32, tag="ot")
                        nc.vector.scalar_tensor_tensor(
                            out=ot[:], in0=pse[bi][:, :], scalar=S3,
                            in1=x2[:, tt, eo:eo + 512], op0=ALU.mult, op1=ALU.add)
                        nc.sync.dma_start(
                            out=d["out"].ap()[tt * 128:(tt + 1) * 128, eo:eo + 512],
                            in_=ot[:])


# ---------------- host side ----------------

NPF8 = ml_dtypes.float8_e4m3  # TRN FP8_EXP4 semantics (bias 7, max 240)


def _f8_scale(w):
    """Power-of-two scale mapping amax into (64, 128] — fp8's sweet spot."""
    amax = float(np.abs(w).max())
    if amax == 0.0:
        return 1.0
    return float(2.0 ** np.floor(np.log2(128.0 / amax)))


def _tile_lhs_f8(wq):
    """Quantized [K, M] -> [MT, 128, KT, 128] (per-m-tile lhsT blocks)."""
    K, M = wq.shape
    t = wq.reshape(K // 128, 128, M // 128, 128).transpose(2, 1, 0, 3)
    return np.ascontiguousarray(t)


def _rows_f8(wq):
    """Quantized [K, N] -> [KT, 128, N] (k-partitioned moving layout)."""
    K, N = wq.shape
    return np.ascontiguousarray(wq.reshape(K // 128, 128, N))


def own_rows(cfg: Cfg, r):
    """Row indices (within the batch) owned by core half r: i-tiles {2j+r}."""
    tiles = [2 * it + r for it in range(cfg.RT)]
    return np.concatenate([np.arange(t * 128, (t + 1) * 128) for t in tiles])


def prepare_core_inputs(inputs, cfg: Cfg, b, r):
    E, T, R, ET, RT = cfg.E, cfg.T, cfg.R, cfg.ET, cfg.RT
    x = np.asarray(inputs["x"])
    rows = own_rows(cfg, r)
    xb = np.ascontiguousarray(x[b]).astype(np.float32)
    x_own = np.ascontiguousarray(xb[rows])
    im = {
        "x_b": xb,
        "x_own": x_own,
        "xT8_in": np.ascontiguousarray(xb.T).reshape(ET, 128, T).astype(NPF8),
        "xo8_in": np.ascontiguousarray(x_own.T).reshape(ET, 128, R).astype(NPF8),
        "ident_in": np.eye(128, dtype=ml_dtypes.bfloat16),
    }
    md = np.empty((RT, 128, 256), np.float32)
    for it in range(RT):
        i_glob = rows[it * 128:(it + 1) * 128]
        j_glob = 256 * it + np.arange(256)
        md[it] = np.where(j_glob[None, :] <= i_glob[:, None], 0.0, -1e30)
    im["mask_d"] = md.astype(ml_dtypes.bfloat16)
    return im


def prepare_shared_weights(inputs, cfg: Cfg):
    """Quantize/tile/scale weights; fold the LN affines into the downstream
    matmuls:  (n*w + b) @ W + c  ==  n @ (diag(w) W) + (b @ W + c).
    The folded-LN colsum corrections (nck/ncq/ncv/nc1) are computed from the
    QUANTIZED weights so the on-device mean subtraction is exact."""
    ln1_w, ln1_b = np.asarray(inputs["ln1_w"]), np.asarray(inputs["ln1_b"])
    ln2_w, ln2_b = np.asarray(inputs["ln2_w"]), np.asarray(inputs["ln2_b"])
    Wq, Wk, Wv = (np.asarray(inputs[k]) for k in ("Wq", "Wk", "Wv"))
    W1 = np.asarray(inputs["W1"])
    wq_e = ln1_w[:, None] * Wq
    wk_e = ln1_w[:, None] * Wk
    wv_e = ln1_w[:, None] * Wv
    bq_e = ln1_b @ Wq + np.asarray(inputs["bq"])
    bk_e = ln1_b @ Wk + np.asarray(inputs["bk"])
    bv_e = ln1_b @ Wv + np.asarray(inputs["bv"])
    w1_e = ln2_w[:, None] * W1
    b1_e = ln2_b @ W1 + np.asarray(inputs["b1"])
    W2a, W2b, W3 = (np.asarray(inputs[k]) for k in ("W2a", "W2b", "W3"))

    def quant(w):
        s = _f8_scale(w)
        return (w * s).astype(NPF8), s

    wq_q, s_q = quant(wq_e)
    wk_q, s_k = quant(wk_e)
    wv_q, s_v = quant(wv_e)
    w1_q, s_1 = quant(w1_e)
    w2a_q, s_2a = quant(W2a)
    w2b_q, s_2b = quant(W2b)
    w3_q, s_3 = quant(W3)

    def ncsum(wq_, s):
        return (-wq_.astype(np.float32).sum(axis=0) / s).astype(np.float32)

    return {
        "wqt": _tile_lhs_f8(wq_q),
        "wkt": _tile_lhs_f8(wk_q),
        "wv": _rows_f8(wv_q),
        "bq": bq_e.astype(np.float32), "bk": bk_e.astype(np.float32),
        "bv": bv_e.astype(np.float32),
        "w1t": _tile_lhs_f8(w1_q),
        "b1": b1_e.astype(np.float32),
        "w2at": _tile_lhs_f8(w2a_q),
        "w2bt": _tile_lhs_f8(w2b_q),
        "w3m": _rows_f8(w3_q),
        "b2a": np.asarray(inputs["b2a"]).astype(np.float32),
        "b2b": np.asarray(inputs["b2b"]).astype(np.float32),
        "b3": np.asarray(inputs["b3"]).astype(np.float32),
        "scales": np.array([1 / s_q, 1 / s_k, 1 / s_v, 1 / s_1,
                            1 / s_2a, 1 / s_2b, 1 / s_3, 0.0], np.float32),
        "nck": ncsum(wk_q, s_k),
        "ncq": ncsum(wq_q, s_q),
        "ncv": ncsum(wv_q, s_v),
        "nc1": ncsum(w1_q, s_1),
    }


_PROGRAM_CACHE = {}


def get_program(cfg: Cfg, reps: int = 1):
    key = (cfg.E, cfg.H, cfg.T, cfg.R, reps)
    if key not in _PROGRAM_CACHE:
        _PROGRAM_CACHE[key] = build_program(cfg, reps=reps)
    return _PROGRAM_CACHE[key]


def run(inputs, cfg: Cfg, trace=False):
    nc = get_program(cfg)
    shared = prepare_shared_weights(inputs, cfg)
    in_maps = []
    for c in range(N_CORES):
        b, r = c // 2, c % 2
        im = prepare_core_inputs(inputs, cfg, b, r)
        im.update(shared)
        in_maps.append(im)
    res = run_bass_kernel_spmd(nc, in_maps, core_ids=list(range(N_CORES)),
                               trace=trace)
    B = np.asarray(inputs["x"]).shape[0]
    outp = np.empty((B, cfg.T, cfg.E), np.float32)
    for c in range(N_CORES):
        b, r = c // 2, c % 2
        outp[b][own_rows(cfg, r)] = res.results[c]["out"]
    return outp, res


def _build_sharded_exec(nc, in_maps):
    """Mirror bass2jax.run_bass_via_pjrt but return a reusable timed runner."""
    import jax
    from jax.sharding import Mesh, PartitionSpec, NamedSharding
    from jax.experimental.shard_map import shard_map
    import concourse.mybir as mb
    from concourse import bass2jax

    bass2jax.install_neuronx_cc_hook()
    n_cores = len(in_maps)
    partition_name = (nc.partition_id_tensor.name
                      if nc.partition_id_tensor is not None else None)
    in_names, out_names, out_avals, zero_outs = [], [], [], []
    for alloc in nc.m.functions[0].allocations:
        if not isinstance(alloc, mb.MemoryLocationSet):
            continue
        name = alloc.memorylocations[0].name
        if alloc.kind == "ExternalInput":
            if name != partition_name:
                in_names.append(name)
        elif alloc.kind == "ExternalOutput":
            out_names.append(name)
            shape = tuple(alloc.tensor_shape)
            dtype = mb.dt.np(alloc.dtype)
            out_avals.append(jax.core.ShapedArray(shape, dtype))
            zero_outs.append(np.zeros(shape, dtype))
    n_params = len(in_names)
    n_outs = len(out_avals)
    all_names = in_names + out_names
    if partition_name is not None:
        all_names = all_names + [partition_name]

    def _call_once(params, zouts):
        operands = list(params) + list(zouts)
        if partition_name is not None:
            operands.append(bass2jax.partition_id_tensor())
        outs = bass2jax._bass_exec_p.bind(
            *operands,
            out_avals=tuple(out_avals),
            in_names=tuple(all_names),
            out_names=tuple(out_names),
            lowering_input_output_aliases=(),
            sim_require_finite=True,
            sim_require_nnan=True,
            nc=nc,
        )
        return tuple(outs)

    def make_body(chain):
        def _body(*args):
            params = args[:n_params]
            outs = args[n_params:]
            for _ in range(chain):
                outs = _call_once(params, outs)
            return tuple(outs)
        return _body

    devices = jax.devices()[:n_cores]
    mesh = Mesh(np.asarray(devices), ("core",))
    in_specs = (PartitionSpec("core"),) * (n_params + n_outs)
    out_specs = (PartitionSpec("core"),) * n_outs
    donate = tuple(range(n_params, n_params + n_outs))

    def make_sharded(chain):
        return jax.jit(
            shard_map(make_body(chain), mesh=mesh, in_specs=in_specs,
                      out_specs=out_specs, check_rep=False),
            donate_argnums=donate, keep_unused=True)

    sharded = make_sharded(1)

    sh = NamedSharding(mesh, PartitionSpec("core"))
    concat_in = [
        jax.device_put(
            np.concatenate([np.asarray(in_maps[c][nm]) for c in range(n_cores)],
                           axis=0), sh)
        for nm in in_names
    ]

    def make_zeros():
        return [jax.device_put(
            np.zeros((n_cores * z.shape[0], *z.shape[1:]), z.dtype), sh)
            for z in zero_outs]

    _jit_cache = {1: sharded}

    def runner(chain=1, nruns=1):
        if chain not in _jit_cache:
            _jit_cache[chain] = make_sharded(chain)
        fn = _jit_cache[chain]
        all_zs = [make_zeros() for _ in range(nruns)]
        for zs in all_zs:
            for z in zs:
                z.block_until_ready()
        t0 = time.perf_counter()
        outs_l = [fn(*concat_in, *zs) for zs in all_zs]
        for outs in outs_l:
            for o in outs:
                o.block_until_ready()
        return time.perf_counter() - t0, outs_l[-1]

    return runner, out_names


def _make_in_maps(inputs, cfg: Cfg):
    shared = prepare_shared_weights(inputs, cfg)
    in_maps = []
    for c in range(N_CORES):
        b, r = c // 2, c % 2
        im = prepare_core_inputs(inputs, cfg, b, r)
        im.update(shared)
        in_maps.append(im)
    return in_maps


def time_exec(inputs, cfg: Cfg, iters=8, reps=3):
    """Per-execution device time via a NEFF containing `reps` unrolled copies
    of the kernel body, differenced against reps=1 to cancel the ~80 ms axon
    dispatch round-trip.  Returns (per_exec_estimate, t1_list, tk_list)."""
    in_maps = _make_in_maps(inputs, cfg)
    r1, _ = _build_sharded_exec(get_program(cfg, reps=1), in_maps)
    rk, _ = _build_sharded_exec(get_program(cfg, reps=reps), in_maps)
    r1(); rk()  # warm both
    t1s, tks = [], []
    for _ in range(iters):
        t1, _ = r1()
        tk, _ = rk()
        t1s.append(t1)
        tks.append(tk)
    med = (np.median(tks) - np.median(t1s)) / (reps - 1)
    return med, t1s, tks


def kernel(**inputs) -> np.ndarray:
    cfg = Cfg(E=1024, H=4096, T=2048, R=1024)
    outp, _ = run(inputs, cfg)
    return outp


# revision 35
# speedup vs baseline: 1.5512x; 1.5512x over previous
"""Trainium2 Bass kernel for a pre-LN transformer block (nn_BaseBlock).

Reference computation (per batch b, fp32):
    h   = LN1(x); k,q,v = h@Wk+bk, h@Wq+bq, h@Wv+bv
    sim = (k @ q^T)/sqrt(E)  (causal tril mask), att = softmax(sim) @ v
    x2  = x + att
    h2  = LN2(x2)
    f   = gelu(gelu(gelu(h2@W1+b1)@W2a+b2a)@W2b+b2b)@W3 + b3
    out = x2 + f

Sharding over 8 cores: core c handles batch b=c//2, row half r=c%2
(i-tiles {2*it + r} of that batch, so the padded causal extent profile is
core-independent).  All matmuls run in fp8e4 with DoubleRow perf mode
(two 128-deep k-tiles contracted per pass); PSUM accumulation is fp32.
Weights are host-scaled by per-tensor powers of two into fp8's sweet spot;
the descale factors ride in a small `scales` input consumed as per-partition
ACT/DVE scale operands, so the compiled program is weight-independent.

LN1 runs feature-major: the host supplies x transposed (xT), token-major
stats are computed on the ACT engine, routed through a DRAM roundtrip into
row-broadcast tiles, and the normalize is two DVE passes straight into the
fp8 feature-major activation tile — no PE transposes for LN1.  Softmax-P
and LN2(h2) still transpose on the PE (bf16, PSUM bounce).

The causal mask enters only through a tiny per-core diagonal-block input
(mask_d, [RT,128,256]); fully-open score blocks skip masking entirely and
exp() reads PSUM directly.  The residual stream x2 stays resident in SBUF
in fp32 (no DRAM spill).
"""

import time

import numpy as np
import ml_dtypes

import concourse.bass as bass
import concourse.mybir as mybir
from concourse import bacc
import concourse.tile as tile
from concourse.bass_utils import run_bass_kernel_spmd

F32 = mybir.dt.float32
BF16 = mybir.dt.bfloat16
F8 = mybir.dt.float8e4
F8E5 = mybir.dt.float8e5
AF = mybir.ActivationFunctionType
ALU = mybir.AluOpType
AX = mybir.AxisListType
DR = mybir.MatmulPerfMode.DoubleRow

EPS = 1e-5
N_CORES = 8


class Cfg:
    def __init__(self, E=1024, H=4096, T=2048, R=1024):
        self.E, self.H, self.T, self.R = E, H, T, R
        self.ET, self.HT, self.CT, self.RT = E // 128, H // 128, T // 128, R // 128
        self.scale = 1.0 / np.sqrt(E)


def _blocks(total, bs=512):
    return [(o, min(bs, total - o)) for o in range(0, total, bs)]


def build_program(cfg: Cfg, reps: int = 1):
    """Build the SPMD Bass program (one core's view).

    reps>1 wraps the body in unrolled copies — used only for timing
    (amortizes the ~80ms axon dispatch round-trip over reps executions).
    """
    E, H, T, R = cfg.E, cfg.H, cfg.T, cfg.R
    ET, HT, CT, RT = cfg.ET, cfg.HT, cfg.CT, cfg.RT

    nc = bacc.Bacc("TRN2", target_bir_lowering=False, debug=False,
                   num_devices=N_CORES)

    # ---- DRAM I/O ----
    xT8_in = nc.dram_tensor("xT8_in", [ET, 128, T], F8, kind="ExternalInput")
    xo8_in = nc.dram_tensor("xo8_in", [ET, 128, R], F8, kind="ExternalInput")
    x_b = nc.dram_tensor("x_b", [T, E], F32, kind="ExternalInput")
    x_own = nc.dram_tensor("x_own", [R, E], F32, kind="ExternalInput")
    mask_d = nc.dram_tensor("mask_d", [RT, 128, 256], BF16, kind="ExternalInput")
    wqt = nc.dram_tensor("wqt", [ET, 128, ET, 128], F8, kind="ExternalInput")
    wkt = nc.dram_tensor("wkt", [ET, 128, ET, 128], F8, kind="ExternalInput")
    wv = nc.dram_tensor("wv", [ET, 128, E], F8, kind="ExternalInput")
    bq = nc.dram_tensor("bq", [E], F32, kind="ExternalInput")
    bk = nc.dram_tensor("bk", [E], F32, kind="ExternalInput")
    bv = nc.dram_tensor("bv", [E], F32, kind="ExternalInput")
    w1t = nc.dram_tensor("w1t", [HT, 128, ET, 128], F8, kind="ExternalInput")
    w2at = nc.dram_tensor("w2at", [HT, 128, HT, 128], F8, kind="ExternalInput")
    w2bt = nc.dram_tensor("w2bt", [HT, 128, HT, 128], F8, kind="ExternalInput")
    w3m = nc.dram_tensor("w3m", [HT, 128, E], F8, kind="ExternalInput")
    b1 = nc.dram_tensor("b1", [H], F32, kind="ExternalInput")
    b2a = nc.dram_tensor("b2a", [H], F32, kind="ExternalInput")
    b2b = nc.dram_tensor("b2b", [H], F32, kind="ExternalInput")
    b3 = nc.dram_tensor("b3", [E], F32, kind="ExternalInput")
    scales = nc.dram_tensor("scales", [8], F32, kind="ExternalInput")
    ident_in = nc.dram_tensor("ident_in", [128, 128], BF16, kind="ExternalInput")
    nck = nc.dram_tensor("nck", [E], F32, kind="ExternalInput")
    ncq = nc.dram_tensor("ncq", [E], F32, kind="ExternalInput")
    ncv = nc.dram_tensor("ncv", [E], F32, kind="ExternalInput")
    nc1 = nc.dram_tensor("nc1", [H], F32, kind="ExternalInput")
    stat_ri = nc.dram_tensor("stat_ri", [T], F32, kind="Internal")
    stat_r2 = nc.dram_tensor("stat_r2", [T], F32, kind="Internal")
    stat_rio = nc.dram_tensor("stat_rio", [R], F32, kind="Internal")
    stat_r2o = nc.dram_tensor("stat_r2o", [R], F32, kind="Internal")
    stat_ri2 = nc.dram_tensor("stat_ri2", [R], F32, kind="Internal")
    stat_r22 = nc.dram_tensor("stat_r22", [R], F32, kind="Internal")
    out = nc.dram_tensor("out", [R, E], F32, kind="ExternalOutput")

    d = locals()
    with tile.TileContext(nc) as tc:
        for _ in range(reps):
            _emit(tc, cfg, d)
    nc.compile()
    return nc


def _emit(tc, cfg, d):
    nc = tc.nc
    E, H, T, R = cfg.E, cfg.H, cfg.T, cfg.R
    ET, HT, CT, RT = cfg.ET, cfg.HT, cfg.CT, cfg.RT
    c32 = float(cfg.scale)
    KP_E = ET // 2   # DoubleRow k-pair count for E contraction
    KP_H = HT // 2

    import contextlib
    ctx = contextlib.ExitStack()
    with ctx:
        consts = ctx.enter_context(tc.tile_pool(name="consts", bufs=1))
        mm_ps = ctx.enter_context(tc.tile_pool(name="mm_ps", bufs=6, space="PSUM"))
        tr_ps = ctx.enter_context(tc.tile_pool(name="tr_ps", bufs=2, space="PSUM"))
        stp = ctx.enter_context(tc.tile_pool(name="ln_stats", bufs=4))
        fxp = ctx.enter_context(tc.tile_pool(name="fixup", bufs=2))

        eps_t = consts.tile([128, 1], F32)
        nc.vector.memset(eps_t[:], EPS)
        ident = consts.tile([128, 128], BF16)
        nc.sync.dma_start(out=ident[:], in_=d["ident_in"].ap())
        neg1 = consts.tile([128, 1], F32)
        nc.vector.memset(neg1[:], -1.0)

        def bcast(name, dr, width=None):
            w = width or dr.shape[0]
            t = consts.tile([128, w], F32, tag=name)
            src = dr.ap()
            src_b = bass.AP(tensor=src.tensor, offset=src.offset,
                            ap=[[0, 128]] + list(src.ap))
            nc.sync.dma_start(out=t[:], in_=src_b)
            return t

        def cols(name, dr, nt):
            t = consts.tile([128, nt], F32, tag=name)
            nc.sync.dma_start(out=t[:], in_=dr.ap().rearrange("(t p) -> p t", p=128))
            return t

        sc = bcast("sc", d["scales"], width=8)
        SQ, SK, SV = sc[:, 0:1], sc[:, 1:2], sc[:, 2:3]
        S1, S2A, S2B, S3 = sc[:, 3:4], sc[:, 4:5], sc[:, 5:6], sc[:, 6:7]
        bq_c = cols("bq", d["bq"], ET)
        bk_c = cols("bk", d["bk"], ET)
        bv_bc = bcast("bv", d["bv"])
        b3_bc = bcast("b3", d["b3"])
        b1_c = cols("b1", d["b1"], HT)
        b2a_c = cols("b2a", d["b2a"], HT)
        b2b_c = cols("b2b", d["b2b"], HT)
        nck_c = cols("nck", d["nck"], ET)
        ncq_c = cols("ncq", d["ncq"], ET)
        ncv_bc = bcast("ncv", d["ncv"])
        nc1_c = cols("nc1", d["nc1"], HT)

        def tile_stats(src_ap, ri_slot, r2_slot):
            """Token-major LN stats of one [128, E] tile.

            ri = 1/sqrt(var+eps), r2 = mu*ri — the two per-token factors the
            folded-LN fixup needs (LN itself never materializes on-chip)."""
            scr = stp.tile([128, E], BF16, tag="scr")
            s1 = stp.tile([128, 1], F32, tag="s1")
            nc.scalar.activation(out=scr[:], in_=src_ap, func=AF.Copy, bias=0.0,
                                 scale=1.0, accum_out=s1[:])
            s2 = stp.tile([128, 1], F32, tag="s2")
            nc.scalar.activation(out=scr[:], in_=src_ap, func=AF.Square,
                                 accum_out=s2[:])
            mu = stp.tile([128, 1], F32, tag="mu_s")
            nc.scalar.mul(out=mu[:], in_=s1[:], mul=1.0 / E)
            mu2 = stp.tile([128, 1], F32, tag="mu2")
            nc.vector.tensor_mul(out=mu2[:], in0=mu[:], in1=mu[:])
            var = stp.tile([128, 1], F32, tag="var")
            nc.vector.scalar_tensor_tensor(out=var[:], in0=s2[:], scalar=1.0 / E,
                                           in1=mu2[:], op0=ALU.mult,
                                           op1=ALU.subtract)
            sd = stp.tile([128, 1], F32, tag="sd")
            nc.scalar.activation(out=sd[:], in_=var[:], func=AF.Sqrt,
                                 bias=eps_t[:], scale=1.0)
            nc.vector.reciprocal(out=ri_slot, in_=sd[:])
            nc.vector.tensor_mul(out=r2_slot, in0=mu[:], in1=ri_slot)

        x2_pool = ctx.enter_context(tc.tile_pool(name="x2", bufs=1))
        x2 = x2_pool.tile([128, RT, E], F32)  # residual stream (own rows)
        h2T_pool = ctx.enter_context(tc.tile_pool(name="h2T_pool", bufs=1))
        h2T = h2T_pool.tile([128, ET, R], F8, tag="h2T")
        ri2_b = h2T_pool.tile([128, R], F32, tag="ri2_b")
        r2_2b = h2T_pool.tile([128, R], F32, tag="r2_2b")

        # ================= attention block =================
        with tc.tile_pool(name="attn_big", bufs=1) as abig:
            xT8 = abig.tile([128, ET, T], F8, tag="xT8")
            xo8 = abig.tile([128, ET, R], F8, tag="xo8")

            qkvp = tc.tile_pool(name="qkvp", bufs=1)
            qkv_pool = qkvp.__enter__()
            statp = tc.tile_pool(name="statp", bufs=1)
            stat_pool = statp.__enter__()

            # x arrives pre-quantized to fp8 from the host (layout prep):
            # 3 MB across both HWDGE queues unblocks the projections in ~5 us
            for kt in range(ET):
                eng = (nc.sync, nc.scalar, nc.gpsimd)[kt % 3]
                eng.dma_start(out=xT8[:, kt, :], in_=d["xT8_in"].ap()[kt])

            rio_all = stat_pool.tile([128, RT], F32, tag="rio_all")
            r2o_all = stat_pool.tile([128, RT], F32, tag="r2o_all")
            ri_all = stat_pool.tile([128, CT], F32, tag="ri_all")
            r2_all = stat_pool.tile([128, CT], F32, tag="r2_all")

            def brow(name, dr, w):
                t = stat_pool.tile([128, w], F32, tag=name, name="brow_t")
                src = dr.ap()
                src_b = bass.AP(tensor=src.tensor, offset=src.offset,
                                ap=[[0, 128]] + list(src.ap))
                nc.sync.dma_start(out=t[:], in_=src_b)
                return t

            qT8 = qkv_pool.tile([128, ET, T], F8, tag="qT8")
            kT8o = qkv_pool.tile([128, ET, R], F8, tag="kT8o")
            vtm = qkv_pool.tile([128, CT, E], F8, tag="vtm")  # token-major v

            def fixup_fm(ps, dst, ri_row, r2_row, ncol, bcol, S, func):
                """Feature-major eviction with folded LN:
                dst = func(S*ps*ri − c_m*r2 + b_m)   (exact LN algebra)."""
                o1 = fxp.tile([128, 512], F32, tag="fx1", name="fx_t")
                nc.vector.scalar_tensor_tensor(out=o1[:], in0=ps, scalar=S,
                                               in1=ri_row, op0=ALU.mult,
                                               op1=ALU.mult)
                o2 = fxp.tile([128, 512], F32, tag="fx2", name="fx_t")
                nc.vector.scalar_tensor_tensor(out=o2[:], in0=r2_row,
                                               scalar=ncol, in1=o1[:],
                                               op0=ALU.mult, op1=ALU.add)
                nc.scalar.activation(out=dst, in_=o2[:], func=func,
                                     bias=bcol, scale=1.0)

            # ---- v (full ctx, token-major), LN stats pipelined per tile ----
            with tc.tile_pool(name="wv_pool", bufs=1) as wvp, \
                 tc.tile_pool(name="ln1w", bufs=3) as lnw:
                wv_sb = wvp.tile([128, ET, E], F8)
                for kt in range(ET):
                    nc.scalar.dma_start(out=wv_sb[:, kt, :], in_=d["wv"].ap()[kt])
                for tt in range(CT):
                    xt = lnw.tile([128, E], F32, tag="xt", bufs=2)
                    nc.gpsimd.dma_start(out=xt[:],
                                        in_=d["x_b"].ap()[tt * 128:(tt + 1) * 128, :])
                    tile_stats(xt[:], ri_all[:, tt:tt + 1], r2_all[:, tt:tt + 1])
                    pss = [mm_ps.tile([128, 512], F32, tag="mm", name="mm_ps_t")
                           for _ in range(2)]
                    for kp in range(KP_E):
                        for bi, eo in enumerate((0, 512)):
                            nc.tensor.matmul(
                                pss[bi][:, :],
                                xT8[:, 2 * kp:2 * kp + 2, tt * 128:(tt + 1) * 128],
                                wv_sb[:, 2 * kp:2 * kp + 2, eo:eo + 512],
                                start=(kp == 0), stop=(kp == KP_E - 1),
                                perf_mode=DR)
                    for bi, eo in enumerate((0, 512)):
                        o1 = fxp.tile([128, 512], F32, tag="fx1", name="fx_t")
                        nc.vector.tensor_scalar(out=o1[:], in0=pss[bi][:, :],
                                                scalar1=SV,
                                                scalar2=ri_all[:, tt:tt + 1],
                                                op0=ALU.mult, op1=ALU.mult)
                        o2 = fxp.tile([128, 512], F32, tag="fx2", name="fx_t")
                        nc.vector.scalar_tensor_tensor(
                            out=o2[:], in0=ncv_bc[:, eo:eo + 512],
                            scalar=r2_all[:, tt:tt + 1], in1=o1[:],
                            op0=ALU.mult, op1=ALU.add)
                        nc.vector.tensor_tensor(out=vtm[:, tt, eo:eo + 512],
                                                in0=o2[:],
                                                in1=bv_bc[:, eo:eo + 512],
                                                op=ALU.add)
                nc.sync.dma_start(
                    out=d["stat_ri"].ap().rearrange("(t p) -> p t", p=128),
                    in_=ri_all[:])
                nc.sync.dma_start(
                    out=d["stat_r2"].ap().rearrange("(t p) -> p t", p=128),
                    in_=r2_all[:])
                ri_b = brow("ri_b", d["stat_ri"], T)
                r2_b = brow("r2_b", d["stat_r2"], T)

                # ---- q (full ctx), feature-major ----
                with tc.tile_pool(name="wq_s", bufs=2) as wqs:
                    for mt in range(ET):
                        wq_mt = wqs.tile([128, ET, 128], F8, tag="wq_mt")
                        nc.scalar.dma_start(out=wq_mt[:], in_=d["wqt"].ap()[mt])
                        pss = [mm_ps.tile([128, 512], F32, tag="mm",
                                          name="mm_ps_t") for _ in range(4)]
                        for kp in range(KP_E):
                            for bi in range(4):
                                nc.tensor.matmul(
                                    pss[bi][:, :], wq_mt[:, 2 * kp:2 * kp + 2, :],
                                    xT8[:, 2 * kp:2 * kp + 2,
                                        bi * 512:(bi + 1) * 512],
                                    start=(kp == 0), stop=(kp == KP_E - 1),
                                    perf_mode=DR)
                        for bi in range(4):
                            fixup_fm(pss[bi][:, :],
                                     qT8[:, mt, bi * 512:(bi + 1) * 512],
                                     ri_b[:, bi * 512:(bi + 1) * 512],
                                     r2_b[:, bi * 512:(bi + 1) * 512],
                                     ncq_c[:, mt:mt + 1], bq_c[:, mt:mt + 1],
                                     SQ, AF.Identity)

                # own rows (k-path): pre-quantized input, stats, roundtrip
                for kt in range(ET):
                    eng = nc.sync if kt % 2 == 0 else nc.scalar
                    eng.dma_start(out=xo8[:, kt, :], in_=d["xo8_in"].ap()[kt])
                for it in range(RT):
                    xt = lnw.tile([128, E], F32, tag="xt", bufs=2)
                    nc.gpsimd.dma_start(out=xt[:],
                                        in_=d["x_own"].ap()[it * 128:(it + 1) * 128, :])
                    tile_stats(xt[:], rio_all[:, it:it + 1], r2o_all[:, it:it + 1])
                nc.sync.dma_start(
                    out=d["stat_rio"].ap().rearrange("(t p) -> p t", p=128),
                    in_=rio_all[:])
                nc.sync.dma_start(
                    out=d["stat_r2o"].ap().rearrange("(t p) -> p t", p=128),
                    in_=r2o_all[:])
                ri_bo = brow("ri_bo", d["stat_rio"], R)
                r2_bo = brow("r2_bo", d["stat_r2o"], R)

                # ---- k (own rows), feature-major ----
                with tc.tile_pool(name="wk_s", bufs=2) as wks:
                    for mt in range(ET):
                        wk_mt = wks.tile([128, ET, 128], F8, tag="wk_mt")
                        nc.scalar.dma_start(out=wk_mt[:], in_=d["wkt"].ap()[mt])
                        pss = [mm_ps.tile([128, 512], F32, tag="mm",
                                          name="mm_ps_t") for _ in range(2)]
                        for kp in range(KP_E):
                            for bi, ro in enumerate((0, 512)):
                                nc.tensor.matmul(
                                    pss[bi][:, :], wk_mt[:, 2 * kp:2 * kp + 2, :],
                                    xo8[:, 2 * kp:2 * kp + 2, ro:ro + 512],
                                    start=(kp == 0), stop=(kp == KP_E - 1),
                                    perf_mode=DR)
                        for bi, ro in enumerate((0, 512)):
                            fixup_fm(pss[bi][:, :], kT8o[:, mt, ro:ro + 512],
                                     ri_bo[:, ro:ro + 512], r2_bo[:, ro:ro + 512],
                                     nck_c[:, mt:mt + 1], bk_c[:, mt:mt + 1],
                                     SK, AF.Identity)

            statp.__exit__(None, None, None)

            # ---- attention rows (own i-tiles), software-pipelined ----
            # Core r owns batch i-tiles {2*it + r}; padded causal extent
            # ext(it) = 2*(it+1) j-tiles is core-independent.  Only the two
            # diagonal j-tiles need masking (mask_d input); earlier blocks are
            # fully open and exp() reads the score PSUM directly.  No
            # max-subtraction: |sim/32| <= ~11 keeps exp in fp32/bf16 range.
            # Scores of i-tile it+1 are emitted before AV of it so the PE works
            # while the pT XBAR-DMA transposes of it are in flight.
            with tc.tile_pool(name="at_p", bufs=2) as pp, \
                 tc.tile_pool(name="at_misc", bufs=3) as msc, \
                 tc.tile_pool(name="at_md", bufs=2) as mdp, \
                 tc.tile_pool(name="ln2", bufs=2) as l2p:
                ri2_all = l2p.tile([128, RT], F32, tag="ri2_all", bufs=1)
                r22_all = l2p.tile([128, RT], F32, tag="r22_all", bufs=1)
                s1_all = l2p.tile([128, RT], F32, tag="s1_all", bufs=1)
                s2_all = l2p.tile([128, RT], F32, tag="s2_all", bufs=1)
                def scores_phase(it):
                    ext = 2 * (it + 1)
                    ncols = ext * 128
                    blks = _blocks(ncols)
                    nblk = len(blks)
                    md = mdp.tile([128, 256], BF16, tag="md", name="at_t")
                    nc.sync.dma_start(out=md[:], in_=d["mask_d"].ap()[it])
                    pss = [mm_ps.tile([128, 512], F32, tag="mm", name="mm_ps_t")
                           for _ in range(nblk)]
                    for kp in range(KP_E):
                        for bi, (jo, jn) in enumerate(blks):
                            nc.tensor.matmul(
                                pss[bi][:, :jn],
                                kT8o[:, 2 * kp:2 * kp + 2, it * 128:(it + 1) * 128],
                                qT8[:, 2 * kp:2 * kp + 2, jo:jo + jn],
                                start=(kp == 0), stop=(kp == KP_E - 1),
                                perf_mode=DR)
                    pbf = pp.tile([128, T], BF16, tag="pbf", name="at_t")
                    lacc = msc.tile([128, 8], F32, tag="lacc", name="at_t")
                    ns = 0
                    for bi, (jo, jn) in enumerate(blks):
                        last = (bi == nblk - 1)
                        jn_open = jn - 256 if last else jn
                        if jn_open > 0:
                            nc.scalar.activation(
                                out=pbf[:, jo:jo + jn_open],
                                in_=pss[bi][:, :jn_open], func=AF.Exp,
                                scale=c32, bias=neg1[:],
                                accum_out=lacc[:, ns:ns + 1])
                            ns += 1
                        if last:
                            simd = msc.tile([128, 256], F32, tag="simd",
                                            name="at_t")
                            nc.vector.tensor_tensor(out=simd[:],
                                                    in0=pss[bi][:, jn_open:jn],
                                                    in1=md[:], op=ALU.add)
                            nc.scalar.activation(
                                out=pbf[:, ncols - 256:ncols], in_=simd[:],
                                func=AF.Exp, scale=c32, bias=neg1[:],
                                accum_out=lacc[:, ns:ns + 1])
                            ns += 1
                    lrow = msc.tile([128, 1], F32, tag="lrow", name="at_t")
                    nc.vector.tensor_reduce(out=lrow[:], in_=lacc[:, :ns],
                                            axis=AX.X, op=ALU.add)
                    linv = msc.tile([128, 1], F32, tag="linv", name="at_t")
                    nc.vector.reciprocal(out=linv[:], in_=lrow[:])
                    return ext, pbf, linv

                def transpose_phase(ext, pbf):
                    pT = pp.tile([128, T], F8E5, tag="pT", name="at_t")
                    for jt in range(ext):
                        tp = tr_ps.tile([128, 128], BF16, tag="tr", name="tr_t")
                        nc.tensor.transpose(tp[:], pbf[:, jt * 128:(jt + 1) * 128],
                                            ident[:])
                        nc.vector.tensor_copy(out=pT[:, jt * 128:(jt + 1) * 128],
                                              in_=tp[:])
                    return pT

                def av_phase(it, ext, pT, linv):
                    xo = msc.tile([128, E], F32, tag="xo", bufs=2, name="at_t")
                    nc.gpsimd.dma_start(out=xo[:],
                                        in_=d["x_own"].ap()[it * 128:(it + 1) * 128, :])
                    pse = [mm_ps.tile([128, 512], F32, tag="mm", name="mm_ps_t")
                           for _ in range(2)]
                    for jp in range(ext // 2):
                        pT_pair = pT[:, jp * 256:(jp + 1) * 256].rearrange(
                            "p (two c) -> p two c", two=2)
                        for bi, eo in enumerate((0, 512)):
                            nc.tensor.matmul(
                                pse[bi][:, :], pT_pair,
                                vtm[:, 2 * jp:2 * jp + 2, eo:eo + 512],
                                start=(jp == 0), stop=(jp == ext // 2 - 1),
                                perf_mode=DR)
                    for bi, eo in enumerate((0, 512)):
                        nc.vector.scalar_tensor_tensor(
                            out=x2[:, it, eo:eo + 512], in0=pse[bi][:, :],
                            scalar=linv[:], in1=xo[:, eo:eo + 512],
                            op0=ALU.mult, op1=ALU.add)
                    # LN2 for this row-tile, fused so DVE/ACT/XBAR do it while
                    # the PE continues with the next i-tile's scores
                    x2bf = l2p.tile([128, E], BF16, tag="x2bf", name="l2_t")
                    nc.vector.tensor_copy(out=x2bf[:], in_=x2[:, it, :])
                    h2bf = l2p.tile([128, E], BF16, tag="h2bf", name="l2_t")
                    for et in range(ET):
                        eng = nc.sync if et % 2 == 0 else nc.scalar
                        eng.dma_start(out=h2bf[:, et * 128:(et + 1) * 128],
                                      in_=x2bf[:, et * 128:(et + 1) * 128],
                                      transpose=True)
                    nc.vector.tensor_copy(
                        out=h2T[:, :, it * 128:(it + 1) * 128],
                        in_=h2bf[:].rearrange("p (et c) -> p et c", c=128))
                    scr2 = stp.tile([128, E], BF16, tag="scr2", name="l2_t")
                    nc.scalar.activation(out=scr2[:], in_=x2[:, it, :],
                                         func=AF.Copy, bias=0.0, scale=1.0,
                                         accum_out=s1_all[:, it:it + 1])
                    scr3 = stp.tile([128, E], BF16, tag="scr3", name="l2_t")
                    nc.scalar.activation(out=scr3[:], in_=x2[:, it, :],
                                         func=AF.Square,
                                         accum_out=s2_all[:, it:it + 1])
                    nc.vector.tensor_tensor(out=x2[:, it, :], in0=x2[:, it, :],
                                            in1=b3_bc[:], op=ALU.add)

                pend = None
                for it in range(RT):
                    ext, pbf, linv = scores_phase(it)
                    if pend is not None:
                        av_phase(pend[0], *pend[1])
                    pT = transpose_phase(ext, pbf)
                    pend = (it, (ext, pT, linv))
                av_phase(pend[0], *pend[1])
                mu8 = stp.tile([128, RT], F32, tag="mu8", name="l2_t")
                nc.scalar.mul(out=mu8[:], in_=s1_all[:], mul=1.0 / E)
                mu28 = stp.tile([128, RT], F32, tag="mu28", name="l2_t")
                nc.vector.tensor_mul(out=mu28[:], in0=mu8[:], in1=mu8[:])
                var8 = stp.tile([128, RT], F32, tag="var8", name="l2_t")
                nc.vector.scalar_tensor_tensor(out=var8[:], in0=s2_all[:],
                                               scalar=1.0 / E, in1=mu28[:],
                                               op0=ALU.mult, op1=ALU.subtract)
                sd8 = stp.tile([128, RT], F32, tag="sd8", name="l2_t")
                nc.scalar.activation(out=sd8[:], in_=var8[:], func=AF.Sqrt,
                                     bias=eps_t[:], scale=1.0)
                nc.vector.reciprocal(out=ri2_all[:], in_=sd8[:])
                nc.vector.tensor_mul(out=r22_all[:], in0=mu8[:], in1=ri2_all[:])
                nc.sync.dma_start(
                    out=d["stat_ri2"].ap().rearrange("(t p) -> p t", p=128),
                    in_=ri2_all[:])
                nc.sync.dma_start(
                    out=d["stat_r22"].ap().rearrange("(t p) -> p t", p=128),
                    in_=r22_all[:])
                srcap = d["stat_ri2"].ap()
                nc.sync.dma_start(out=ri2_b[:], in_=bass.AP(
                    tensor=srcap.tensor, offset=srcap.offset,
                    ap=[[0, 128]] + list(srcap.ap)))
                srcap = d["stat_r22"].ap()
                nc.sync.dma_start(out=r2_2b[:], in_=bass.AP(
                    tensor=srcap.tensor, offset=srcap.offset,
                    ap=[[0, 128]] + list(srcap.ap)))
            qkvp.__exit__(None, None, None)

        # ================= MLP block =================
        with tc.tile_pool(name="gx", bufs=1) as gxp, \
             tc.tile_pool(name="mlp_ws", bufs=1) as ws:
            g1T = gxp.tile([128, HT, R], F8, tag="gx")
            for mt in range(HT):
                w1_mt = ws.tile([128, ET, 128], F8, tag="w1_mt", bufs=3)
                nc.scalar.dma_start(out=w1_mt[:], in_=d["w1t"].ap()[mt])
                pss = [mm_ps.tile([128, 512], F32, tag="mm", name="mm_ps_t")
                       for _ in range(2)]
                for kp in range(KP_E):
                    for bi, ro in enumerate((0, 512)):
                        nc.tensor.matmul(
                            pss[bi][:, :], w1_mt[:, 2 * kp:2 * kp + 2, :],
                            h2T[:, 2 * kp:2 * kp + 2, ro:ro + 512],
                            start=(kp == 0), stop=(kp == KP_E - 1), perf_mode=DR)
                for bi, ro in enumerate((0, 512)):
                    fixup_fm(pss[bi][:, :], g1T[:, mt, ro:ro + 512],
                             ri2_b[:, ro:ro + 512], r2_2b[:, ro:ro + 512],
                             nc1_c[:, mt:mt + 1], b1_c[:, mt:mt + 1],
                             S1, AF.Gelu)

            with tc.tile_pool(name="g2", bufs=1) as g2p:
                g2T = g2p.tile([128, HT, R], F8, tag="g2")
                for mt in range(HT):
                    w2_mt = ws.tile([128, HT, 128], F8, tag="w2a_mt", bufs=3)
                    nc.scalar.dma_start(out=w2_mt[:], in_=d["w2at"].ap()[mt])
                    pss = [mm_ps.tile([128, 512], F32, tag="mm", name="mm_ps_t")
                           for _ in range(2)]
                    for kp in range(KP_H):
                        for bi, ro in enumerate((0, 512)):
                            nc.tensor.matmul(
                                pss[bi][:, :], w2_mt[:, 2 * kp:2 * kp + 2, :],
                                g1T[:, 2 * kp:2 * kp + 2, ro:ro + 512],
                                start=(kp == 0), stop=(kp == KP_H - 1),
                                perf_mode=DR)
                    for bi, ro in enumerate((0, 512)):
                        nc.scalar.activation(out=g2T[:, mt, ro:ro + 512],
                                             in_=pss[bi][:, :], func=AF.Gelu,
                                             bias=b2a_c[:, mt:mt + 1], scale=S2A)

                g3T = gxp.tile([128, HT, R], F8, tag="gx")
                for mt in range(HT):
                    w2_mt = ws.tile([128, HT, 128], F8, tag="w2b_mt", bufs=3)
                    nc.scalar.dma_start(out=w2_mt[:], in_=d["w2bt"].ap()[mt])
                    pss = [mm_ps.tile([128, 512], F32, tag="mm", name="mm_ps_t")
                           for _ in range(2)]
                    for kp in range(KP_H):
                        for bi, ro in enumerate((0, 512)):
                            nc.tensor.matmul(
                                pss[bi][:, :], w2_mt[:, 2 * kp:2 * kp + 2, :],
                                g2T[:, 2 * kp:2 * kp + 2, ro:ro + 512],
                                start=(kp == 0), stop=(kp == KP_H - 1),
                                perf_mode=DR)
                    for bi, ro in enumerate((0, 512)):
                        nc.scalar.activation(out=g3T[:, mt, ro:ro + 512],
                                             in_=pss[bi][:, :], func=AF.Gelu,
                                             bias=b2b_c[:, mt:mt + 1], scale=S2B)

            # ---- f = g3 @ W3 (+b3 already in x2); out = x2 + f ----
            with tc.tile_pool(name="w3p", bufs=1) as w3p, \
                 tc.tile_pool(name="outp", bufs=2) as op:
                w3_sb = w3p.tile([128, HT, E], F8)
                for kt in range(HT):
                    nc.scalar.dma_start(out=w3_sb[:, kt, :], in_=d["w3m"].ap()[kt])
                for tt in range(RT):
                    pse = [mm_ps.tile([128, 512], F32, tag="mm", name="mm_ps_t")
                           for _ in range(2)]
                    for kp in range(KP_H):
                        for bi, eo in enumerate((0, 512)):
                            nc.tensor.matmul(
                                pse[bi][:, :],
                                g3T[:, 2 * kp:2 * kp + 2, tt * 128:(tt + 1) * 128],
                                w3_sb[:, 2 * kp:2 * kp + 2, eo:eo + 512],
                                start=(kp == 0), stop=(kp == KP_H - 1),
                                perf_mode=DR)
                    for bi, eo in enumerate((0, 512)):
                        ot = op.tile([128, 512], F32, tag="ot")
                        nc.vector.scalar_tensor_tensor(
                            out=ot[:], in0=pse[bi][:, :], scalar=S3,
                            in1=x2[:, tt, eo:eo + 512], op0=ALU.mult, op1=ALU.add)
                        eng = nc.sync if bi == 0 else nc.gpsimd
                        eng.dma_start(
                            out=d["out"].ap()[tt * 128:(tt + 1) * 128, eo:eo + 512],
                            in_=ot[:])


# ---------------- host side ----------------

NPF8 = ml_dtypes.float8_e4m3  # TRN FP8_EXP4 semantics (bias 7, max 240)


def _f8_scale(w):
    """Power-of-two scale mapping amax into (64, 128] — fp8's sweet spot."""
    amax = float(np.abs(w).max())
    if amax == 0.0:
        return 1.0
    return float(2.0 ** np.floor(np.log2(128.0 / amax)))


def _tile_lhs_f8(wq):
    """Quantized [K, M] -> [MT, 128, KT, 128] (per-m-tile lhsT blocks)."""
    K, M = wq.shape
    t = wq.reshape(K // 128, 128, M // 128, 128).transpose(2, 1, 0, 3)
    return np.ascontiguousarray(t)


def _rows_f8(wq):
    """Quantized [K, N] -> [KT, 128, N] (k-partitioned moving layout)."""
    K, N = wq.shape
    return np.ascontiguousarray(wq.reshape(K // 128, 128, N))


def own_rows(cfg: Cfg, r):
    """Row indices (within the batch) owned by core half r: i-tiles {2j+r}."""
    tiles = [2 * it + r for it in range(cfg.RT)]
    return np.concatenate([np.arange(t * 128, (t + 1) * 128) for t in tiles])


def prepare_core_inputs(inputs, cfg: Cfg, b, r):
    E, T, R, ET, RT = cfg.E, cfg.T, cfg.R, cfg.ET, cfg.RT
    x = np.asarray(inputs["x"])
    rows = own_rows(cfg, r)
    xb = np.ascontiguousarray(x[b]).astype(np.float32)
    x_own = np.ascontiguousarray(xb[rows])
    im = {
        "x_b": xb,
        "x_own": x_own,
        "xT8_in": np.ascontiguousarray(xb.T).reshape(ET, 128, T).astype(NPF8),
        "xo8_in": np.ascontiguousarray(x_own.T).reshape(ET, 128, R).astype(NPF8),
        "ident_in": np.eye(128, dtype=ml_dtypes.bfloat16),
    }
    md = np.empty((RT, 128, 256), np.float32)
    for it in range(RT):
        i_glob = rows[it * 128:(it + 1) * 128]
        j_glob = 256 * it + np.arange(256)
        md[it] = np.where(j_glob[None, :] <= i_glob[:, None], 0.0, -1e30)
    im["mask_d"] = md.astype(ml_dtypes.bfloat16)
    return im


def prepare_shared_weights(inputs, cfg: Cfg):
    """Quantize/tile/scale weights; fold the LN affines into the downstream
    matmuls:  (n*w + b) @ W + c  ==  n @ (diag(w) W) + (b @ W + c).
    The folded-LN colsum corrections (nck/ncq/ncv/nc1) are computed from the
    QUANTIZED weights so the on-device mean subtraction is exact."""
    ln1_w, ln1_b = np.asarray(inputs["ln1_w"]), np.asarray(inputs["ln1_b"])
    ln2_w, ln2_b = np.asarray(inputs["ln2_w"]), np.asarray(inputs["ln2_b"])
    Wq, Wk, Wv = (np.asarray(inputs[k]) for k in ("Wq", "Wk", "Wv"))
    W1 = np.asarray(inputs["W1"])
    wq_e = ln1_w[:, None] * Wq
    wk_e = ln1_w[:, None] * Wk
    wv_e = ln1_w[:, None] * Wv
    bq_e = ln1_b @ Wq + np.asarray(inputs["bq"])
    bk_e = ln1_b @ Wk + np.asarray(inputs["bk"])
    bv_e = ln1_b @ Wv + np.asarray(inputs["bv"])
    w1_e = ln2_w[:, None] * W1
    b1_e = ln2_b @ W1 + np.asarray(inputs["b1"])
    W2a, W2b, W3 = (np.asarray(inputs[k]) for k in ("W2a", "W2b", "W3"))

    def quant(w):
        s = _f8_scale(w)
        return (w * s).astype(NPF8), s

    wq_q, s_q = quant(wq_e)
    wk_q, s_k = quant(wk_e)
    wv_q, s_v = quant(wv_e)
    w1_q, s_1 = quant(w1_e)
    w2a_q, s_2a = quant(W2a)
    w2b_q, s_2b = quant(W2b)
    w3_q, s_3 = quant(W3)

    def ncsum(wq_, s):
        return (-wq_.astype(np.float32).sum(axis=0) / s).astype(np.float32)

    return {
        "wqt": _tile_lhs_f8(wq_q),
        "wkt": _tile_lhs_f8(wk_q),
        "wv": _rows_f8(wv_q),
        "bq": bq_e.astype(np.float32), "bk": bk_e.astype(np.float32),
        "bv": bv_e.astype(np.float32),
        "w1t": _tile_lhs_f8(w1_q),
        "b1": b1_e.astype(np.float32),
        "w2at": _tile_lhs_f8(w2a_q),
        "w2bt": _tile_lhs_f8(w2b_q),
        "w3m": _rows_f8(w3_q),
        "b2a": np.asarray(inputs["b2a"]).astype(np.float32),
        "b2b": np.asarray(inputs["b2b"]).astype(np.float32),
        "b3": np.asarray(inputs["b3"]).astype(np.float32),
        "scales": np.array([1 / s_q, 1 / s_k, 1 / s_v, 1 / s_1,
                            1 / s_2a, 1 / s_2b, 1 / s_3, 0.0], np.float32),
        "nck": ncsum(wk_q, s_k),
        "ncq": ncsum(wq_q, s_q),
        "ncv": ncsum(wv_q, s_v),
        "nc1": ncsum(w1_q, s_1),
    }


_PROGRAM_CACHE = {}


def get_program(cfg: Cfg, reps: int = 1):
    key = (cfg.E, cfg.H, cfg.T, cfg.R, reps)
    if key not in _PROGRAM_CACHE:
        _PROGRAM_CACHE[key] = build_program(cfg, reps=reps)
    return _PROGRAM_CACHE[key]


def run(inputs, cfg: Cfg, trace=False):
    nc = get_program(cfg)
    shared = prepare_shared_weights(inputs, cfg)
    in_maps = []
    for c in range(N_CORES):
        b, r = c // 2, c % 2
        im = prepare_core_inputs(inputs, cfg, b, r)
        im.update(shared)
        in_maps.append(im)
    res = run_bass_kernel_spmd(nc, in_maps, core_ids=list(range(N_CORES)),
                               trace=trace)
    B = np.asarray(inputs["x"]).shape[0]
    outp = np.empty((B, cfg.T, cfg.E), np.float32)
    for c in range(N_CORES):
        b, r = c // 2, c % 2
        outp[b][own_rows(cfg, r)] = res.results[c]["out"]
    return outp, res


def _build_sharded_exec(nc, in_maps):
    """Mirror bass2jax.run_bass_via_pjrt but return a reusable timed runner."""
    import jax
    from jax.sharding import Mesh, PartitionSpec, NamedSharding
    from jax.experimental.shard_map import shard_map
    import concourse.mybir as mb
    from concourse import bass2jax

    bass2jax.install_neuronx_cc_hook()
    n_cores = len(in_maps)
    partition_name = (nc.partition_id_tensor.name
                      if nc.partition_id_tensor is not None else None)
    in_names, out_names, out_avals, zero_outs = [], [], [], []
    for alloc in nc.m.functions[0].allocations:
        if not isinstance(alloc, mb.MemoryLocationSet):
            continue
        name = alloc.memorylocations[0].name
        if alloc.kind == "ExternalInput":
            if name != partition_name:
                in_names.append(name)
        elif alloc.kind == "ExternalOutput":
            out_names.append(name)
            shape = tuple(alloc.tensor_shape)
            dtype = mb.dt.np(alloc.dtype)
            out_avals.append(jax.core.ShapedArray(shape, dtype))
            zero_outs.append(np.zeros(shape, dtype))
    n_params = len(in_names)
    n_outs = len(out_avals)
    all_names = in_names + out_names
    if partition_name is not None:
        all_names = all_names + [partition_name]

    def _call_once(params, zouts):
        operands = list(params) + list(zouts)
        if partition_name is not None:
            operands.append(bass2jax.partition_id_tensor())
        outs = bass2jax._bass_exec_p.bind(
            *operands,
            out_avals=tuple(out_avals),
            in_names=tuple(all_names),
            out_names=tuple(out_names),
            lowering_input_output_aliases=(),
            sim_require_finite=True,
            sim_require_nnan=True,
            nc=nc,
        )
        return tuple(outs)

    def make_body(chain):
        def _body(*args):
            params = args[:n_params]
            outs = args[n_params:]
            for _ in range(chain):
                outs = _call_once(params, outs)
            return tuple(outs)
        return _body

    devices = jax.devices()[:n_cores]
    mesh = Mesh(np.asarray(devices), ("core",))
    in_specs = (PartitionSpec("core"),) * (n_params + n_outs)
    out_specs = (PartitionSpec("core"),) * n_outs
    donate = tuple(range(n_params, n_params + n_outs))

    def make_sharded(chain):
        return jax.jit(
            shard_map(make_body(chain), mesh=mesh, in_specs=in_specs,
                      out_specs=out_specs, check_rep=False),
            donate_argnums=donate, keep_unused=True)

    sharded = make_sharded(1)

    sh = NamedSharding(mesh, PartitionSpec("core"))
    concat_in = [
        jax.device_put(
            np.concatenate([np.asarray(in_maps[c][nm]) for c in range(n_cores)],
                           axis=0), sh)
        for nm in in_names
    ]

    def make_zeros():
        return [jax.device_put(
            np.zeros((n_cores * z.shape[0], *z.shape[1:]), z.dtype), sh)
            for z in zero_outs]

    _jit_cache = {1: sharded}

    def runner(chain=1, nruns=1):
        if chain not in _jit_cache:
            _jit_cache[chain] = make_sharded(chain)
        fn = _jit_cache[chain]
        all_zs = [make_zeros() for _ in range(nruns)]
        for zs in all_zs:
            for z in zs:
                z.block_until_ready()
        t0 = time.perf_counter()
        outs_l = [fn(*concat_in, *zs) for zs in all_zs]
        for outs in outs_l:
            for o in outs:
                o.block_until_ready()
        return time.perf_counter() - t0, outs_l[-1]

    return runner, out_names


def _make_in_maps(inputs, cfg: Cfg):
    shared = prepare_shared_weights(inputs, cfg)
    in_maps = []
    for c in range(N_CORES):
        b, r = c // 2, c % 2
        im = prepare_core_inputs(inputs, cfg, b, r)
        im.update(shared)
        in_maps.append(im)
    return in_maps


def time_exec(inputs, cfg: Cfg, iters=8, reps=3):
    """Per-execution device time via a NEFF containing `reps` unrolled copies
    of the kernel body, differenced against reps=1 to cancel the ~80 ms axon
    dispatch round-trip.  Returns (per_exec_estimate, t1_list, tk_list)."""
    in_maps = _make_in_maps(inputs, cfg)
    r1, _ = _build_sharded_exec(get_program(cfg, reps=1), in_maps)
    rk, _ = _build_sharded_exec(get_program(cfg, reps=reps), in_maps)
    r1(); rk()  # warm both
    t1s, tks = [], []
    for _ in range(iters):
        t1, _ = r1()
        tk, _ = rk()
        t1s.append(t1)
        tks.append(tk)
    med = (np.median(tks) - np.median(t1s)) / (reps - 1)
    return med, t1s, tks


def kernel(**inputs) -> np.ndarray:
    cfg = Cfg(E=1024, H=4096, T=2048, R=1024)
    outp, _ = run(inputs, cfg)
    return outp


# revision 36
# speedup vs baseline: 1.8091x; 1.1663x over previous
"""Trainium2 Bass kernel for a pre-LN transformer block (nn_BaseBlock).

Reference computation (per batch b, fp32):
    h   = LN1(x); k,q,v = h@Wk+bk, h@Wq+bq, h@Wv+bv
    sim = (k @ q^T)/sqrt(E)  (causal tril mask), att = softmax(sim) @ v
    x2  = x + att
    h2  = LN2(x2)
    f   = gelu(gelu(gelu(h2@W1+b1)@W2a+b2a)@W2b+b2b)@W3 + b3
    out = x2 + f

Sharding over 8 cores: core c handles batch b=c//2, row half r=c%2
(i-tiles {2*it + r} of that batch, so the padded causal extent profile is
core-independent).  All matmuls run in fp8e4 with DoubleRow perf mode
(two 128-deep k-tiles contracted per pass); PSUM accumulation is fp32.
Weights are host-scaled by per-tensor powers of two into fp8's sweet spot;
the descale factors ride in a small `scales` input consumed as per-partition
ACT/DVE scale operands, so the compiled program is weight-independent.

LN1 runs feature-major: the host supplies x transposed (xT), token-major
stats are computed on the ACT engine, routed through a DRAM roundtrip into
row-broadcast tiles, and the normalize is two DVE passes straight into the
fp8 feature-major activation tile — no PE transposes for LN1.  Softmax-P
and LN2(h2) still transpose on the PE (bf16, PSUM bounce).

The causal mask enters only through a tiny per-core diagonal-block input
(mask_d, [RT,128,256]); fully-open score blocks skip masking entirely and
exp() reads PSUM directly.  The residual stream x2 stays resident in SBUF
in fp32 (no DRAM spill).
"""

import time

import numpy as np
import ml_dtypes

import concourse.bass as bass
import concourse.mybir as mybir
from concourse import bacc
import concourse.tile as tile
from concourse.bass_utils import run_bass_kernel_spmd

F32 = mybir.dt.float32
BF16 = mybir.dt.bfloat16
F8 = mybir.dt.float8e4
F8E5 = mybir.dt.float8e5
AF = mybir.ActivationFunctionType
ALU = mybir.AluOpType
AX = mybir.AxisListType
DR = mybir.MatmulPerfMode.DoubleRow

EPS = 1e-5
N_CORES = 8


class Cfg:
    def __init__(self, E=1024, H=4096, T=2048, R=1024):
        self.E, self.H, self.T, self.R = E, H, T, R
        self.ET, self.HT, self.CT, self.RT = E // 128, H // 128, T // 128, R // 128
        self.scale = 1.0 / np.sqrt(E)


def _blocks(total, bs=512):
    return [(o, min(bs, total - o)) for o in range(0, total, bs)]


def build_program(cfg: Cfg, reps: int = 1):
    """Build the SPMD Bass program (one core's view).

    reps>1 wraps the body in unrolled copies — used only for timing
    (amortizes the ~80ms axon dispatch round-trip over reps executions).
    """
    E, H, T, R = cfg.E, cfg.H, cfg.T, cfg.R
    ET, HT, CT, RT = cfg.ET, cfg.HT, cfg.CT, cfg.RT

    nc = bacc.Bacc("TRN2", target_bir_lowering=False, debug=False,
                   num_devices=N_CORES)

    # ---- DRAM I/O ----
    xT8_in = nc.dram_tensor("xT8_in", [ET, 128, T], F8, kind="ExternalInput")
    xo8_in = nc.dram_tensor("xo8_in", [ET, 128, R], F8, kind="ExternalInput")
    x_b = nc.dram_tensor("x_b", [T, E], F32, kind="ExternalInput")
    x_own = nc.dram_tensor("x_own", [R, E], F32, kind="ExternalInput")
    mask_d = nc.dram_tensor("mask_d", [RT, 128, 256], BF16, kind="ExternalInput")
    wqt = nc.dram_tensor("wqt", [ET, 128, ET, 128], F8, kind="ExternalInput")
    wkt = nc.dram_tensor("wkt", [ET, 128, ET, 128], F8, kind="ExternalInput")
    wv = nc.dram_tensor("wv", [ET, 128, E], F8, kind="ExternalInput")
    bq = nc.dram_tensor("bq", [E], F32, kind="ExternalInput")
    bk = nc.dram_tensor("bk", [E], F32, kind="ExternalInput")
    bv = nc.dram_tensor("bv", [E], F32, kind="ExternalInput")
    w1t = nc.dram_tensor("w1t", [HT, 128, ET, 128], F8, kind="ExternalInput")
    w2at = nc.dram_tensor("w2at", [HT, 128, HT, 128], F8, kind="ExternalInput")
    w2bt = nc.dram_tensor("w2bt", [HT, 128, HT, 128], F8, kind="ExternalInput")
    w3m = nc.dram_tensor("w3m", [HT, 128, E], F8, kind="ExternalInput")
    b1 = nc.dram_tensor("b1", [H], F32, kind="ExternalInput")
    b2a = nc.dram_tensor("b2a", [H], F32, kind="ExternalInput")
    b2b = nc.dram_tensor("b2b", [H], F32, kind="ExternalInput")
    b3 = nc.dram_tensor("b3", [E], F32, kind="ExternalInput")
    scales = nc.dram_tensor("scales", [8], F32, kind="ExternalInput")
    ident_in = nc.dram_tensor("ident_in", [128, 128], BF16, kind="ExternalInput")
    nck = nc.dram_tensor("nck", [E], F32, kind="ExternalInput")
    ncq = nc.dram_tensor("ncq", [E], F32, kind="ExternalInput")
    ncv = nc.dram_tensor("ncv", [E], F32, kind="ExternalInput")
    nc1 = nc.dram_tensor("nc1", [H], F32, kind="ExternalInput")
    stat_ri = nc.dram_tensor("stat_ri", [T], F32, kind="Internal")
    stat_r2 = nc.dram_tensor("stat_r2", [T], F32, kind="Internal")
    stat_rio = nc.dram_tensor("stat_rio", [R], F32, kind="Internal")
    stat_r2o = nc.dram_tensor("stat_r2o", [R], F32, kind="Internal")
    stat_ri2 = nc.dram_tensor("stat_ri2", [R], F32, kind="Internal")
    stat_r22 = nc.dram_tensor("stat_r22", [R], F32, kind="Internal")
    out = nc.dram_tensor("out", [R, E], F32, kind="ExternalOutput")

    d = locals()
    with tile.TileContext(nc) as tc:
        for _ in range(reps):
            _emit(tc, cfg, d)
    nc.compile()
    return nc


def _emit(tc, cfg, d):
    nc = tc.nc
    E, H, T, R = cfg.E, cfg.H, cfg.T, cfg.R
    ET, HT, CT, RT = cfg.ET, cfg.HT, cfg.CT, cfg.RT
    c32 = float(cfg.scale)
    KP_E = ET // 2   # DoubleRow k-pair count for E contraction
    KP_H = HT // 2

    import contextlib
    ctx = contextlib.ExitStack()
    with ctx:
        consts = ctx.enter_context(tc.tile_pool(name="consts", bufs=1))
        mm_ps = ctx.enter_context(tc.tile_pool(name="mm_ps", bufs=6, space="PSUM"))
        tr_ps = ctx.enter_context(tc.tile_pool(name="tr_ps", bufs=2, space="PSUM"))
        stp = ctx.enter_context(tc.tile_pool(name="ln_stats", bufs=4))
        fxp = ctx.enter_context(tc.tile_pool(name="fixup", bufs=2))

        eps_t = consts.tile([128, 1], F32)
        nc.vector.memset(eps_t[:], EPS)
        ident = consts.tile([128, 128], BF16)
        nc.sync.dma_start(out=ident[:], in_=d["ident_in"].ap())
        neg1 = consts.tile([128, 1], F32)
        nc.vector.memset(neg1[:], -1.0)

        def bcast(name, dr, width=None):
            w = width or dr.shape[0]
            t = consts.tile([128, w], F32, tag=name)
            src = dr.ap()
            src_b = bass.AP(tensor=src.tensor, offset=src.offset,
                            ap=[[0, 128]] + list(src.ap))
            nc.sync.dma_start(out=t[:], in_=src_b)
            return t

        def cols(name, dr, nt):
            t = consts.tile([128, nt], F32, tag=name)
            nc.sync.dma_start(out=t[:], in_=dr.ap().rearrange("(t p) -> p t", p=128))
            return t

        sc = bcast("sc", d["scales"], width=8)
        SQ, SK, SV = sc[:, 0:1], sc[:, 1:2], sc[:, 2:3]
        S1, S2A, S2B, S3 = sc[:, 3:4], sc[:, 4:5], sc[:, 5:6], sc[:, 6:7]
        bq_c = cols("bq", d["bq"], ET)
        bk_c = cols("bk", d["bk"], ET)
        bv_bc = bcast("bv", d["bv"])
        b3_bc = bcast("b3", d["b3"])
        b1_c = cols("b1", d["b1"], HT)
        b2a_c = cols("b2a", d["b2a"], HT)
        b2b_c = cols("b2b", d["b2b"], HT)
        nck_c = cols("nck", d["nck"], ET)
        ncq_c = cols("ncq", d["ncq"], ET)
        ncv_bc = bcast("ncv", d["ncv"])
        nc1_c = cols("nc1", d["nc1"], HT)

        def tile_stats(src_ap, ri_slot, r2_slot):
            """Token-major LN stats of one [128, E] tile.

            ri = 1/sqrt(var+eps), r2 = mu*ri — the two per-token factors the
            folded-LN fixup needs (LN itself never materializes on-chip)."""
            scr = stp.tile([128, E], BF16, tag="scr")
            s1 = stp.tile([128, 1], F32, tag="s1")
            nc.scalar.activation(out=scr[:], in_=src_ap, func=AF.Copy, bias=0.0,
                                 scale=1.0, accum_out=s1[:])
            s2 = stp.tile([128, 1], F32, tag="s2")
            nc.scalar.activation(out=scr[:], in_=src_ap, func=AF.Square,
                                 accum_out=s2[:])
            mu = stp.tile([128, 1], F32, tag="mu_s")
            nc.scalar.mul(out=mu[:], in_=s1[:], mul=1.0 / E)
            mu2 = stp.tile([128, 1], F32, tag="mu2")
            nc.vector.tensor_mul(out=mu2[:], in0=mu[:], in1=mu[:])
            var = stp.tile([128, 1], F32, tag="var")
            nc.vector.scalar_tensor_tensor(out=var[:], in0=s2[:], scalar=1.0 / E,
                                           in1=mu2[:], op0=ALU.mult,
                                           op1=ALU.subtract)
            sd = stp.tile([128, 1], F32, tag="sd")
            nc.scalar.activation(out=sd[:], in_=var[:], func=AF.Sqrt,
                                 bias=eps_t[:], scale=1.0)
            nc.vector.reciprocal(out=ri_slot, in_=sd[:])
            nc.vector.tensor_mul(out=r2_slot, in0=mu[:], in1=ri_slot)

        x2_pool = ctx.enter_context(tc.tile_pool(name="x2", bufs=1))
        x2 = x2_pool.tile([128, RT, E], F32)  # residual stream (own rows)
        h2T_pool = ctx.enter_context(tc.tile_pool(name="h2T_pool", bufs=1))
        h2T = h2T_pool.tile([128, ET, R], F8, tag="h2T")
        ri2_b = h2T_pool.tile([128, R], F32, tag="ri2_b")
        r2_2b = h2T_pool.tile([128, R], F32, tag="r2_2b")

        # ================= attention block =================
        with tc.tile_pool(name="attn_big", bufs=1) as abig:
            xT8 = abig.tile([128, ET, T], F8, tag="xT8")
            xo8 = abig.tile([128, ET, R], F8, tag="xo8")

            qkvp = tc.tile_pool(name="qkvp", bufs=1)
            qkv_pool = qkvp.__enter__()
            statp = tc.tile_pool(name="statp", bufs=1)
            stat_pool = statp.__enter__()

            # x arrives pre-quantized to fp8 from the host (layout prep):
            # 3 MB across both HWDGE queues unblocks the projections in ~5 us
            for kt in range(ET):
                eng = (nc.sync, nc.scalar, nc.gpsimd)[kt % 3]
                eng.dma_start(out=xT8[:, kt, :], in_=d["xT8_in"].ap()[kt])
            for kt in range(ET):
                eng = (nc.scalar, nc.sync)[kt % 2]
                eng.dma_start(out=xo8[:, kt, :], in_=d["xo8_in"].ap()[kt])

            rio_all = stat_pool.tile([128, RT], F32, tag="rio_all")
            r2o_all = stat_pool.tile([128, RT], F32, tag="r2o_all")
            ri_all = stat_pool.tile([128, CT], F32, tag="ri_all")
            r2_all = stat_pool.tile([128, CT], F32, tag="r2_all")

            def brow(name, dr, w):
                t = stat_pool.tile([128, w], F32, tag=name, name="brow_t")
                src = dr.ap()
                src_b = bass.AP(tensor=src.tensor, offset=src.offset,
                                ap=[[0, 128]] + list(src.ap))
                nc.sync.dma_start(out=t[:], in_=src_b)
                return t

            qT8 = qkv_pool.tile([128, ET, T], F8, tag="qT8")
            kT8o = qkv_pool.tile([128, ET, R], F8, tag="kT8o")
            vtm = qkv_pool.tile([128, CT, E], F8, tag="vtm")  # token-major v

            def fixup_fm(ps, dst, ri_row, r2_row, ncol, bcol, S, func):
                """Feature-major eviction with folded LN:
                dst = func(S*ps*ri − c_m*r2 + b_m)   (exact LN algebra)."""
                o1 = fxp.tile([128, 512], F32, tag="fx1", name="fx_t")
                nc.vector.scalar_tensor_tensor(out=o1[:], in0=ps, scalar=S,
                                               in1=ri_row, op0=ALU.mult,
                                               op1=ALU.mult)
                o2 = fxp.tile([128, 512], F32, tag="fx2", name="fx_t")
                nc.vector.scalar_tensor_tensor(out=o2[:], in0=r2_row,
                                               scalar=ncol, in1=o1[:],
                                               op0=ALU.mult, op1=ALU.add)
                nc.scalar.activation(out=dst, in_=o2[:], func=func,
                                     bias=bcol, scale=1.0)

            # ---- v (full ctx, token-major), LN stats pipelined per tile ----
            with tc.tile_pool(name="wv_pool", bufs=1) as wvp, \
                 tc.tile_pool(name="ln1w", bufs=3) as lnw:
                wv_sb = wvp.tile([128, ET, E], F8)
                for kt in range(ET):
                    nc.scalar.dma_start(out=wv_sb[:, kt, :], in_=d["wv"].ap()[kt])
                def ctx_stats(st):
                    xt = lnw.tile([128, E], F32, tag="xt", bufs=2, name="xt_t")
                    nc.gpsimd.dma_start(out=xt[:],
                                        in_=d["x_b"].ap()[st * 128:(st + 1) * 128, :])
                    tile_stats(xt[:], ri_all[:, st:st + 1], r2_all[:, st:st + 1])
                def own_stats(it):
                    xt = lnw.tile([128, E], F32, tag="xt", bufs=2, name="xt_t")
                    nc.gpsimd.dma_start(out=xt[:],
                                        in_=d["x_own"].ap()[it * 128:(it + 1) * 128, :])
                    tile_stats(xt[:], rio_all[:, it:it + 1], r2o_all[:, it:it + 1])
                for tt in range(CT):
                    # ctx stats run two tiles ahead of the v evictions; own
                    # stats (k-path) fill the second half so the k roundtrip
                    # is ready the moment the q loop drains
                    if tt < CT // 2:
                        ctx_stats(2 * tt)
                        ctx_stats(2 * tt + 1)
                    else:
                        own_stats(tt - CT // 2)
                    pss = [mm_ps.tile([128, 512], F32, tag="mm", name="mm_ps_t")
                           for _ in range(2)]
                    for kp in range(KP_E):
                        for bi, eo in enumerate((0, 512)):
                            nc.tensor.matmul(
                                pss[bi][:, :],
                                xT8[:, 2 * kp:2 * kp + 2, tt * 128:(tt + 1) * 128],
                                wv_sb[:, 2 * kp:2 * kp + 2, eo:eo + 512],
                                start=(kp == 0), stop=(kp == KP_E - 1),
                                perf_mode=DR)
                    for bi, eo in enumerate((0, 512)):
                        o1 = fxp.tile([128, 512], F32, tag="fx1", name="fx_t")
                        nc.vector.tensor_scalar(out=o1[:], in0=pss[bi][:, :],
                                                scalar1=SV,
                                                scalar2=ri_all[:, tt:tt + 1],
                                                op0=ALU.mult, op1=ALU.mult)
                        o2 = fxp.tile([128, 512], F32, tag="fx2", name="fx_t")
                        nc.vector.scalar_tensor_tensor(
                            out=o2[:], in0=ncv_bc[:, eo:eo + 512],
                            scalar=r2_all[:, tt:tt + 1], in1=o1[:],
                            op0=ALU.mult, op1=ALU.add)
                        nc.vector.tensor_tensor(out=vtm[:, tt, eo:eo + 512],
                                                in0=o2[:],
                                                in1=bv_bc[:, eo:eo + 512],
                                                op=ALU.add)
                nc.sync.dma_start(
                    out=d["stat_ri"].ap().rearrange("(t p) -> p t", p=128),
                    in_=ri_all[:])
                nc.sync.dma_start(
                    out=d["stat_r2"].ap().rearrange("(t p) -> p t", p=128),
                    in_=r2_all[:])
                ri_b = brow("ri_b", d["stat_ri"], T)
                r2_b = brow("r2_b", d["stat_r2"], T)

                # own-rows roundtrip (stats computed during the v loop)
                nc.sync.dma_start(
                    out=d["stat_rio"].ap().rearrange("(t p) -> p t", p=128),
                    in_=rio_all[:])
                nc.sync.dma_start(
                    out=d["stat_r2o"].ap().rearrange("(t p) -> p t", p=128),
                    in_=r2o_all[:])
                ri_bo = brow("ri_bo", d["stat_rio"], R)
                r2_bo = brow("r2_bo", d["stat_r2o"], R)

                # ---- q (full ctx), feature-major ----
                with tc.tile_pool(name="wq_s", bufs=2) as wqs:
                    for mt in range(ET):
                        wq_mt = wqs.tile([128, ET, 128], F8, tag="wq_mt", bufs=3)
                        nc.scalar.dma_start(out=wq_mt[:], in_=d["wqt"].ap()[mt])
                        pss = [mm_ps.tile([128, 512], F32, tag="mm",
                                          name="mm_ps_t") for _ in range(4)]
                        for kp in range(KP_E):
                            for bi in range(4):
                                nc.tensor.matmul(
                                    pss[bi][:, :], wq_mt[:, 2 * kp:2 * kp + 2, :],
                                    xT8[:, 2 * kp:2 * kp + 2,
                                        bi * 512:(bi + 1) * 512],
                                    start=(kp == 0), stop=(kp == KP_E - 1),
                                    perf_mode=DR)
                        for bi in range(4):
                            fixup_fm(pss[bi][:, :],
                                     qT8[:, mt, bi * 512:(bi + 1) * 512],
                                     ri_b[:, bi * 512:(bi + 1) * 512],
                                     r2_b[:, bi * 512:(bi + 1) * 512],
                                     ncq_c[:, mt:mt + 1], bq_c[:, mt:mt + 1],
                                     SQ, AF.Identity)

                # ---- k (own rows), feature-major ----
                with tc.tile_pool(name="wk_s", bufs=2) as wks:
                    for mt in range(ET):
                        wk_mt = wks.tile([128, ET, 128], F8, tag="wk_mt", bufs=3)
                        nc.scalar.dma_start(out=wk_mt[:], in_=d["wkt"].ap()[mt])
                        pss = [mm_ps.tile([128, 512], F32, tag="mm",
                                          name="mm_ps_t") for _ in range(2)]
                        for kp in range(KP_E):
                            for bi, ro in enumerate((0, 512)):
                                nc.tensor.matmul(
                                    pss[bi][:, :], wk_mt[:, 2 * kp:2 * kp + 2, :],
                                    xo8[:, 2 * kp:2 * kp + 2, ro:ro + 512],
                                    start=(kp == 0), stop=(kp == KP_E - 1),
                                    perf_mode=DR)
                        for bi, ro in enumerate((0, 512)):
                            fixup_fm(pss[bi][:, :], kT8o[:, mt, ro:ro + 512],
                                     ri_bo[:, ro:ro + 512], r2_bo[:, ro:ro + 512],
                                     nck_c[:, mt:mt + 1], bk_c[:, mt:mt + 1],
                                     SK, AF.Identity)

            statp.__exit__(None, None, None)

            # ---- attention rows (own i-tiles), software-pipelined ----
            # Core r owns batch i-tiles {2*it + r}; padded causal extent
            # ext(it) = 2*(it+1) j-tiles is core-independent.  Only the two
            # diagonal j-tiles need masking (mask_d input); earlier blocks are
            # fully open and exp() reads the score PSUM directly.  No
            # max-subtraction: |sim/32| <= ~11 keeps exp in fp32/bf16 range.
            # Scores of i-tile it+1 are emitted before AV of it so the PE works
            # while the pT XBAR-DMA transposes of it are in flight.
            with tc.tile_pool(name="at_p", bufs=2) as pp, \
                 tc.tile_pool(name="at_misc", bufs=3) as msc, \
                 tc.tile_pool(name="at_md", bufs=2) as mdp, \
                 tc.tile_pool(name="ln2", bufs=2) as l2p:
                ri2_all = l2p.tile([128, RT], F32, tag="ri2_all", bufs=1)
                r22_all = l2p.tile([128, RT], F32, tag="r22_all", bufs=1)
                s1_all = l2p.tile([128, RT], F32, tag="s1_all", bufs=1)
                s2_all = l2p.tile([128, RT], F32, tag="s2_all", bufs=1)
                def scores_phase(it):
                    ext = 2 * (it + 1)
                    ncols = ext * 128
                    blks = _blocks(ncols)
                    nblk = len(blks)
                    md = mdp.tile([128, 256], BF16, tag="md", name="at_t")
                    nc.sync.dma_start(out=md[:], in_=d["mask_d"].ap()[it])
                    pss = [mm_ps.tile([128, 512], F32, tag="mm", name="mm_ps_t")
                           for _ in range(nblk)]
                    for kp in range(KP_E):
                        for bi, (jo, jn) in enumerate(blks):
                            nc.tensor.matmul(
                                pss[bi][:, :jn],
                                kT8o[:, 2 * kp:2 * kp + 2, it * 128:(it + 1) * 128],
                                qT8[:, 2 * kp:2 * kp + 2, jo:jo + jn],
                                start=(kp == 0), stop=(kp == KP_E - 1),
                                perf_mode=DR)
                    pbf = pp.tile([128, T], BF16, tag="pbf", name="at_t")
                    lacc = msc.tile([128, 8], F32, tag="lacc", name="at_t")
                    ns = 0
                    for bi, (jo, jn) in enumerate(blks):
                        last = (bi == nblk - 1)
                        jn_open = jn - 256 if last else jn
                        if jn_open > 0:
                            nc.scalar.activation(
                                out=pbf[:, jo:jo + jn_open],
                                in_=pss[bi][:, :jn_open], func=AF.Exp,
                                scale=c32, bias=neg1[:],
                                accum_out=lacc[:, ns:ns + 1])
                            ns += 1
                        if last:
                            simd = msc.tile([128, 256], F32, tag="simd",
                                            name="at_t")
                            nc.vector.tensor_tensor(out=simd[:],
                                                    in0=pss[bi][:, jn_open:jn],
                                                    in1=md[:], op=ALU.add)
                            nc.scalar.activation(
                                out=pbf[:, ncols - 256:ncols], in_=simd[:],
                                func=AF.Exp, scale=c32, bias=neg1[:],
                                accum_out=lacc[:, ns:ns + 1])
                            ns += 1
                    lrow = msc.tile([128, 1], F32, tag="lrow", name="at_t")
                    nc.vector.tensor_reduce(out=lrow[:], in_=lacc[:, :ns],
                                            axis=AX.X, op=ALU.add)
                    linv = msc.tile([128, 1], F32, tag="linv", name="at_t")
                    nc.vector.reciprocal(out=linv[:], in_=lrow[:])
                    return ext, pbf, linv

                def transpose_phase(ext, pbf):
                    pT = pp.tile([128, T], F8E5, tag="pT", name="at_t")
                    for jt in range(ext):
                        tp = tr_ps.tile([128, 128], BF16, tag="tr", name="tr_t")
                        nc.tensor.transpose(tp[:], pbf[:, jt * 128:(jt + 1) * 128],
                                            ident[:])
                        nc.vector.tensor_copy(out=pT[:, jt * 128:(jt + 1) * 128],
                                              in_=tp[:])
                    return pT

                def av_phase(it, ext, pT, linv):
                    xo = msc.tile([128, E], F32, tag="xo", bufs=2, name="at_t")
                    nc.gpsimd.dma_start(out=xo[:],
                                        in_=d["x_own"].ap()[it * 128:(it + 1) * 128, :])
                    pse = [mm_ps.tile([128, 512], F32, tag="mm", name="mm_ps_t")
                           for _ in range(2)]
                    for jp in range(ext // 2):
                        pT_pair = pT[:, jp * 256:(jp + 1) * 256].rearrange(
                            "p (two c) -> p two c", two=2)
                        for bi, eo in enumerate((0, 512)):
                            nc.tensor.matmul(
                                pse[bi][:, :], pT_pair,
                                vtm[:, 2 * jp:2 * jp + 2, eo:eo + 512],
                                start=(jp == 0), stop=(jp == ext // 2 - 1),
                                perf_mode=DR)
                    for bi, eo in enumerate((0, 512)):
                        nc.vector.scalar_tensor_tensor(
                            out=x2[:, it, eo:eo + 512], in0=pse[bi][:, :],
                            scalar=linv[:], in1=xo[:, eo:eo + 512],
                            op0=ALU.mult, op1=ALU.add)
                    # LN2 for this row-tile, fused so DVE/ACT/XBAR do it while
                    # the PE continues with the next i-tile's scores
                    x2bf = l2p.tile([128, E], BF16, tag="x2bf", name="l2_t")
                    nc.vector.tensor_copy(out=x2bf[:], in_=x2[:, it, :])
                    h2bf = l2p.tile([128, E], BF16, tag="h2bf", name="l2_t")
                    for et in range(ET):
                        eng = nc.sync if et % 2 == 0 else nc.scalar
                        eng.dma_start(out=h2bf[:, et * 128:(et + 1) * 128],
                                      in_=x2bf[:, et * 128:(et + 1) * 128],
                                      transpose=True)
                    nc.vector.tensor_copy(
                        out=h2T[:, :, it * 128:(it + 1) * 128],
                        in_=h2bf[:].rearrange("p (et c) -> p et c", c=128))
                    scr2 = stp.tile([128, E], BF16, tag="scr2", name="l2_t")
                    nc.scalar.activation(out=scr2[:], in_=x2[:, it, :],
                                         func=AF.Copy, bias=0.0, scale=1.0,
                                         accum_out=s1_all[:, it:it + 1])
                    scr3 = stp.tile([128, E], BF16, tag="scr3", name="l2_t")
                    nc.scalar.activation(out=scr3[:], in_=x2[:, it, :],
                                         func=AF.Square,
                                         accum_out=s2_all[:, it:it + 1])
                    nc.vector.tensor_tensor(out=x2[:, it, :], in0=x2[:, it, :],
                                            in1=b3_bc[:], op=ALU.add)

                pend = None
                for it in range(RT):
                    ext, pbf, linv = scores_phase(it)
                    if pend is not None:
                        av_phase(pend[0], *pend[1])
                    pT = transpose_phase(ext, pbf)
                    pend = (it, (ext, pT, linv))
                av_phase(pend[0], *pend[1])
                mu8 = stp.tile([128, RT], F32, tag="mu8", name="l2_t")
                nc.scalar.mul(out=mu8[:], in_=s1_all[:], mul=1.0 / E)
                mu28 = stp.tile([128, RT], F32, tag="mu28", name="l2_t")
                nc.vector.tensor_mul(out=mu28[:], in0=mu8[:], in1=mu8[:])
                var8 = stp.tile([128, RT], F32, tag="var8", name="l2_t")
                nc.vector.scalar_tensor_tensor(out=var8[:], in0=s2_all[:],
                                               scalar=1.0 / E, in1=mu28[:],
                                               op0=ALU.mult, op1=ALU.subtract)
                sd8 = stp.tile([128, RT], F32, tag="sd8", name="l2_t")
                nc.scalar.activation(out=sd8[:], in_=var8[:], func=AF.Sqrt,
                                     bias=eps_t[:], scale=1.0)
                nc.vector.reciprocal(out=ri2_all[:], in_=sd8[:])
                nc.vector.tensor_mul(out=r22_all[:], in0=mu8[:], in1=ri2_all[:])
                nc.sync.dma_start(
                    out=d["stat_ri2"].ap().rearrange("(t p) -> p t", p=128),
                    in_=ri2_all[:])
                nc.sync.dma_start(
                    out=d["stat_r22"].ap().rearrange("(t p) -> p t", p=128),
                    in_=r22_all[:])
                srcap = d["stat_ri2"].ap()
                nc.sync.dma_start(out=ri2_b[:], in_=bass.AP(
                    tensor=srcap.tensor, offset=srcap.offset,
                    ap=[[0, 128]] + list(srcap.ap)))
                srcap = d["stat_r22"].ap()
                nc.sync.dma_start(out=r2_2b[:], in_=bass.AP(
                    tensor=srcap.tensor, offset=srcap.offset,
                    ap=[[0, 128]] + list(srcap.ap)))
            qkvp.__exit__(None, None, None)

        # ================= MLP block =================
        with tc.tile_pool(name="gx", bufs=1) as gxp, \
             tc.tile_pool(name="mlp_ws", bufs=1) as ws:
            g1T = gxp.tile([128, HT, R], F8, tag="gx")
            for mt in range(HT):
                w1_mt = ws.tile([128, ET, 128], F8, tag="w1_mt", bufs=3)
                nc.scalar.dma_start(out=w1_mt[:], in_=d["w1t"].ap()[mt])
                pss = [mm_ps.tile([128, 512], F32, tag="mm", name="mm_ps_t")
                       for _ in range(2)]
                for kp in range(KP_E):
                    for bi, ro in enumerate((0, 512)):
                        nc.tensor.matmul(
                            pss[bi][:, :], w1_mt[:, 2 * kp:2 * kp + 2, :],
                            h2T[:, 2 * kp:2 * kp + 2, ro:ro + 512],
                            start=(kp == 0), stop=(kp == KP_E - 1), perf_mode=DR)
                for bi, ro in enumerate((0, 512)):
                    fixup_fm(pss[bi][:, :], g1T[:, mt, ro:ro + 512],
                             ri2_b[:, ro:ro + 512], r2_2b[:, ro:ro + 512],
                             nc1_c[:, mt:mt + 1], b1_c[:, mt:mt + 1],
                             S1, AF.Gelu)

            with tc.tile_pool(name="g2", bufs=1) as g2p:
                g2T = g2p.tile([128, HT, R], F8, tag="g2")
                for mt in range(HT):
                    w2_mt = ws.tile([128, HT, 128], F8, tag="w2a_mt", bufs=3)
                    nc.scalar.dma_start(out=w2_mt[:], in_=d["w2at"].ap()[mt])
                    pss = [mm_ps.tile([128, 512], F32, tag="mm", name="mm_ps_t")
                           for _ in range(2)]
                    for kp in range(KP_H):
                        for bi, ro in enumerate((0, 512)):
                            nc.tensor.matmul(
                                pss[bi][:, :], w2_mt[:, 2 * kp:2 * kp + 2, :],
                                g1T[:, 2 * kp:2 * kp + 2, ro:ro + 512],
                                start=(kp == 0), stop=(kp == KP_H - 1),
                                perf_mode=DR)
                    for bi, ro in enumerate((0, 512)):
                        nc.scalar.activation(out=g2T[:, mt, ro:ro + 512],
                                             in_=pss[bi][:, :], func=AF.Gelu,
                                             bias=b2a_c[:, mt:mt + 1], scale=S2A)

                g3T = gxp.tile([128, HT, R], F8, tag="gx")
                for mt in range(HT):
                    w2_mt = ws.tile([128, HT, 128], F8, tag="w2b_mt", bufs=3)
                    nc.scalar.dma_start(out=w2_mt[:], in_=d["w2bt"].ap()[mt])
                    pss = [mm_ps.tile([128, 512], F32, tag="mm", name="mm_ps_t")
                           for _ in range(2)]
                    for kp in range(KP_H):
                        for bi, ro in enumerate((0, 512)):
                            nc.tensor.matmul(
                                pss[bi][:, :], w2_mt[:, 2 * kp:2 * kp + 2, :],
                                g2T[:, 2 * kp:2 * kp + 2, ro:ro + 512],
                                start=(kp == 0), stop=(kp == KP_H - 1),
                                perf_mode=DR)
                    for bi, ro in enumerate((0, 512)):
                        nc.scalar.activation(out=g3T[:, mt, ro:ro + 512],
                                             in_=pss[bi][:, :], func=AF.Gelu,
                                             bias=b2b_c[:, mt:mt + 1], scale=S2B)

            # ---- f = g3 @ W3 (+b3 already in x2); out = x2 + f ----
            with tc.tile_pool(name="w3p", bufs=1) as w3p, \
                 tc.tile_pool(name="outp", bufs=2) as op:
                w3_sb = w3p.tile([128, HT, E], F8)
                for kt in range(HT):
                    nc.scalar.dma_start(out=w3_sb[:, kt, :], in_=d["w3m"].ap()[kt])
                for tt in range(RT):
                    pse = [mm_ps.tile([128, 512], F32, tag="mm", name="mm_ps_t")
                           for _ in range(2)]
                    for kp in range(KP_H):
                        for bi, eo in enumerate((0, 512)):
                            nc.tensor.matmul(
                                pse[bi][:, :],
                                g3T[:, 2 * kp:2 * kp + 2, tt * 128:(tt + 1) * 128],
                                w3_sb[:, 2 * kp:2 * kp + 2, eo:eo + 512],
                                start=(kp == 0), stop=(kp == KP_H - 1),
                                perf_mode=DR)
                    for bi, eo in enumerate((0, 512)):
                        ot = op.tile([128, 512], F32, tag="ot")
                        nc.vector.scalar_tensor_tensor(
                            out=ot[:], in0=pse[bi][:, :], scalar=S3,
                            in1=x2[:, tt, eo:eo + 512], op0=ALU.mult, op1=ALU.add)
                        eng = nc.sync if bi == 0 else nc.gpsimd
                        eng.dma_start(
                            out=d["out"].ap()[tt * 128:(tt + 1) * 128, eo:eo + 512],
                            in_=ot[:])


# ---------------- host side ----------------

NPF8 = ml_dtypes.float8_e4m3  # TRN FP8_EXP4 semantics (bias 7, max 240)


def _f8_scale(w):
    """Power-of-two scale mapping amax into (64, 128] — fp8's sweet spot."""
    amax = float(np.abs(w).max())
    if amax == 0.0:
        return 1.0
    return float(2.0 ** np.floor(np.log2(128.0 / amax)))


def _tile_lhs_f8(wq):
    """Quantized [K, M] -> [MT, 128, KT, 128] (per-m-tile lhsT blocks)."""
    K, M = wq.shape
    t = wq.reshape(K // 128, 128, M // 128, 128).transpose(2, 1, 0, 3)
    return np.ascontiguousarray(t)


def _rows_f8(wq):
    """Quantized [K, N] -> [KT, 128, N] (k-partitioned moving layout)."""
    K, N = wq.shape
    return np.ascontiguousarray(wq.reshape(K // 128, 128, N))


def own_rows(cfg: Cfg, r):
    """Row indices (within the batch) owned by core half r: i-tiles {2j+r}."""
    tiles = [2 * it + r for it in range(cfg.RT)]
    return np.concatenate([np.arange(t * 128, (t + 1) * 128) for t in tiles])


def prepare_core_inputs(inputs, cfg: Cfg, b, r):
    E, T, R, ET, RT = cfg.E, cfg.T, cfg.R, cfg.ET, cfg.RT
    x = np.asarray(inputs["x"])
    rows = own_rows(cfg, r)
    xb = np.ascontiguousarray(x[b]).astype(np.float32)
    x_own = np.ascontiguousarray(xb[rows])
    im = {
        "x_b": xb,
        "x_own": x_own,
        "xT8_in": np.ascontiguousarray(xb.T).reshape(ET, 128, T).astype(NPF8),
        "xo8_in": np.ascontiguousarray(x_own.T).reshape(ET, 128, R).astype(NPF8),
        "ident_in": np.eye(128, dtype=ml_dtypes.bfloat16),
    }
    md = np.empty((RT, 128, 256), np.float32)
    for it in range(RT):
        i_glob = rows[it * 128:(it + 1) * 128]
        j_glob = 256 * it + np.arange(256)
        md[it] = np.where(j_glob[None, :] <= i_glob[:, None], 0.0, -1e30)
    im["mask_d"] = md.astype(ml_dtypes.bfloat16)
    return im


def prepare_shared_weights(inputs, cfg: Cfg):
    """Quantize/tile/scale weights; fold the LN affines into the downstream
    matmuls:  (n*w + b) @ W + c  ==  n @ (diag(w) W) + (b @ W + c).
    The folded-LN colsum corrections (nck/ncq/ncv/nc1) are computed from the
    QUANTIZED weights so the on-device mean subtraction is exact."""
    ln1_w, ln1_b = np.asarray(inputs["ln1_w"]), np.asarray(inputs["ln1_b"])
    ln2_w, ln2_b = np.asarray(inputs["ln2_w"]), np.asarray(inputs["ln2_b"])
    Wq, Wk, Wv = (np.asarray(inputs[k]) for k in ("Wq", "Wk", "Wv"))
    W1 = np.asarray(inputs["W1"])
    wq_e = ln1_w[:, None] * Wq
    wk_e = ln1_w[:, None] * Wk
    wv_e = ln1_w[:, None] * Wv
    bq_e = ln1_b @ Wq + np.asarray(inputs["bq"])
    bk_e = ln1_b @ Wk + np.asarray(inputs["bk"])
    bv_e = ln1_b @ Wv + np.asarray(inputs["bv"])
    w1_e = ln2_w[:, None] * W1
    b1_e = ln2_b @ W1 + np.asarray(inputs["b1"])
    W2a, W2b, W3 = (np.asarray(inputs[k]) for k in ("W2a", "W2b", "W3"))

    def quant(w):
        s = _f8_scale(w)
        return (w * s).astype(NPF8), s

    wq_q, s_q = quant(wq_e)
    wk_q, s_k = quant(wk_e)
    wv_q, s_v = quant(wv_e)
    w1_q, s_1 = quant(w1_e)
    w2a_q, s_2a = quant(W2a)
    w2b_q, s_2b = quant(W2b)
    w3_q, s_3 = quant(W3)

    def ncsum(wq_, s):
        return (-wq_.astype(np.float32).sum(axis=0) / s).astype(np.float32)

    return {
        "wqt": _tile_lhs_f8(wq_q),
        "wkt": _tile_lhs_f8(wk_q),
        "wv": _rows_f8(wv_q),
        "bq": bq_e.astype(np.float32), "bk": bk_e.astype(np.float32),
        "bv": bv_e.astype(np.float32),
        "w1t": _tile_lhs_f8(w1_q),
        "b1": b1_e.astype(np.float32),
        "w2at": _tile_lhs_f8(w2a_q),
        "w2bt": _tile_lhs_f8(w2b_q),
        "w3m": _rows_f8(w3_q),
        "b2a": np.asarray(inputs["b2a"]).astype(np.float32),
        "b2b": np.asarray(inputs["b2b"]).astype(np.float32),
        "b3": np.asarray(inputs["b3"]).astype(np.float32),
        "scales": np.array([1 / s_q, 1 / s_k, 1 / s_v, 1 / s_1,
                            1 / s_2a, 1 / s_2b, 1 / s_3, 0.0], np.float32),
        "nck": ncsum(wk_q, s_k),
        "ncq": ncsum(wq_q, s_q),
        "ncv": ncsum(wv_q, s_v),
        "nc1": ncsum(w1_q, s_1),
    }


_PROGRAM_CACHE = {}


def get_program(cfg: Cfg, reps: int = 1):
    key = (cfg.E, cfg.H, cfg.T, cfg.R, reps)
    if key not in _PROGRAM_CACHE:
        _PROGRAM_CACHE[key] = build_program(cfg, reps=reps)
    return _PROGRAM_CACHE[key]


def run(inputs, cfg: Cfg, trace=False):
    nc = get_program(cfg)
    shared = prepare_shared_weights(inputs, cfg)
    in_maps = []
    for c in range(N_CORES):
        b, r = c // 2, c % 2
        im = prepare_core_inputs(inputs, cfg, b, r)
        im.update(shared)
        in_maps.append(im)
    res = run_bass_kernel_spmd(nc, in_maps, core_ids=list(range(N_CORES)),
                               trace=trace)
    B = np.asarray(inputs["x"]).shape[0]
    outp = np.empty((B, cfg.T, cfg.E), np.float32)
    for c in range(N_CORES):
        b, r = c // 2, c % 2
        outp[b][own_rows(cfg, r)] = res.results[c]["out"]
    return outp, res


def _build_sharded_exec(nc, in_maps):
    """Mirror bass2jax.run_bass_via_pjrt but return a reusable timed runner."""
    import jax
    from jax.sharding import Mesh, PartitionSpec, NamedSharding
    from jax.experimental.shard_map import shard_map
    import concourse.mybir as mb
    from concourse import bass2jax

    bass2jax.install_neuronx_cc_hook()
    n_cores = len(in_maps)
    partition_name = (nc.partition_id_tensor.name
                      if nc.partition_id_tensor is not None else None)
    in_names, out_names, out_avals, zero_outs = [], [], [], []
    for alloc in nc.m.functions[0].allocations:
        if not isinstance(alloc, mb.MemoryLocationSet):
            continue
        name = alloc.memorylocations[0].name
        if alloc.kind == "ExternalInput":
            if name != partition_name:
                in_names.append(name)
        elif alloc.kind == "ExternalOutput":
            out_names.append(name)
            shape = tuple(alloc.tensor_shape)
            dtype = mb.dt.np(alloc.dtype)
            out_avals.append(jax.core.ShapedArray(shape, dtype))
            zero_outs.append(np.zeros(shape, dtype))
    n_params = len(in_names)
    n_outs = len(out_avals)
    all_names = in_names + out_names
    if partition_name is not None:
        all_names = all_names + [partition_name]

    def _call_once(params, zouts):
        operands = list(params) + list(zouts)
        if partition_name is not None:
            operands.append(bass2jax.partition_id_tensor())
        outs = bass2jax._bass_exec_p.bind(
            *operands,
            out_avals=tuple(out_avals),
            in_names=tuple(all_names),
            out_names=tuple(out_names),
            lowering_input_output_aliases=(),
            sim_require_finite=True,
            sim_require_nnan=True,
            nc=nc,
        )
        return tuple(outs)

    def make_body(chain):
        def _body(*args):
            params = args[:n_params]
            outs = args[n_params:]
            for _ in range(chain):
                outs = _call_once(params, outs)
            return tuple(outs)
        return _body

    devices = jax.devices()[:n_cores]
    mesh = Mesh(np.asarray(devices), ("core",))
    in_specs = (PartitionSpec("core"),) * (n_params + n_outs)
    out_specs = (PartitionSpec("core"),) * n_outs
    donate = tuple(range(n_params, n_params + n_outs))

    def make_sharded(chain):
        return jax.jit(
            shard_map(make_body(chain), mesh=mesh, in_specs=in_specs,
                      out_specs=out_specs, check_rep=False),
            donate_argnums=donate, keep_unused=True)

    sharded = make_sharded(1)

    sh = NamedSharding(mesh, PartitionSpec("core"))
    concat_in = [
        jax.device_put(
            np.concatenate([np.asarray(in_maps[c][nm]) for c in range(n_cores)],
                           axis=0), sh)
        for nm in in_names
    ]

    def make_zeros():
        return [jax.device_put(
            np.zeros((n_cores * z.shape[0], *z.shape[1:]), z.dtype), sh)
            for z in zero_outs]

    _jit_cache = {1: sharded}

    def runner(chain=1, nruns=1):
        if chain not in _jit_cache:
            _jit_cache[chain] = make_sharded(chain)
        fn = _jit_cache[chain]
        all_zs = [make_zeros() for _ in range(nruns)]
        for zs in all_zs:
            for z in zs:
                z.block_until_ready()
        t0 = time.perf_counter()
        outs_l = [fn(*concat_in, *zs) for zs in all_zs]
        for outs in outs_l:
            for o in outs:
                o.block_until_ready()
        return time.perf_counter() - t0, outs_l[-1]

    return runner, out_names


def _make_in_maps(inputs, cfg: Cfg):
    shared = prepare_shared_weights(inputs, cfg)
    in_maps = []
    for c in range(N_CORES):
        b, r = c // 2, c % 2
        im = prepare_core_inputs(inputs, cfg, b, r)
        im.update(shared)
        in_maps.append(im)
    return in_maps


def time_exec(inputs, cfg: Cfg, iters=8, reps=3):
    """Per-execution device time via a NEFF containing `reps` unrolled copies
    of the kernel body, differenced against reps=1 to cancel the ~80 ms axon
    dispatch round-trip.  Returns (per_exec_estimate, t1_list, tk_list)."""
    in_maps = _make_in_maps(inputs, cfg)
    r1, _ = _build_sharded_exec(get_program(cfg, reps=1), in_maps)
    rk, _ = _build_sharded_exec(get_program(cfg, reps=reps), in_maps)
    r1(); rk()  # warm both
    t1s, tks = [], []
    for _ in range(iters):
        t1, _ = r1()
        tk, _ = rk()
        t1s.append(t1)
        tks.append(tk)
    med = (np.median(tks) - np.median(t1s)) / (reps - 1)
    return med, t1s, tks


def kernel(**inputs) -> np.ndarray:
    cfg = Cfg(E=1024, H=4096, T=2048, R=1024)
    outp, _ = run(inputs, cfg)
    return outp


# revision 39
# speedup vs baseline: 1.8802x; 1.0393x over previous
"""Trainium2 Bass kernel for a pre-LN transformer block (nn_BaseBlock).

Reference computation (per batch b, fp32):
    h   = LN1(x); k,q,v = h@Wk+bk, h@Wq+bq, h@Wv+bv
    sim = (k @ q^T)/sqrt(E)  (causal tril mask), att = softmax(sim) @ v
    x2  = x + att
    h2  = LN2(x2)
    f   = gelu(gelu(gelu(h2@W1+b1)@W2a+b2a)@W2b+b2b)@W3 + b3
    out = x2 + f

Sharding over 8 cores: core c handles batch b=c//2, row half r=c%2
(i-tiles {2*it + r} of that batch, so the padded causal extent profile is
core-independent).  All matmuls run in fp8e4 with DoubleRow perf mode
(two 128-deep k-tiles contracted per pass); PSUM accumulation is fp32.
Weights are host-scaled by per-tensor powers of two into fp8's sweet spot;
the descale factors ride in a small `scales` input consumed as per-partition
ACT/DVE scale operands, so the compiled program is weight-independent.

LN1 runs feature-major: the host supplies x transposed (xT), token-major
stats are computed on the ACT engine, routed through a DRAM roundtrip into
row-broadcast tiles, and the normalize is two DVE passes straight into the
fp8 feature-major activation tile — no PE transposes for LN1.  Softmax-P
and LN2(h2) still transpose on the PE (bf16, PSUM bounce).

The causal mask enters only through a tiny per-core diagonal-block input
(mask_d, [RT,128,256]); fully-open score blocks skip masking entirely and
exp() reads PSUM directly.  The residual stream x2 stays resident in SBUF
in fp32 (no DRAM spill).
"""

import time

import numpy as np
import ml_dtypes

import concourse.bass as bass
import concourse.mybir as mybir
from concourse import bacc
import concourse.tile as tile
from concourse.bass_utils import run_bass_kernel_spmd

F32 = mybir.dt.float32
BF16 = mybir.dt.bfloat16
F8 = mybir.dt.float8e4
F8E5 = mybir.dt.float8e5
AF = mybir.ActivationFunctionType
ALU = mybir.AluOpType
AX = mybir.AxisListType
DR = mybir.MatmulPerfMode.DoubleRow

EPS = 1e-5
N_CORES = 8


class Cfg:
    def __init__(self, E=1024, H=4096, T=2048, R=1024):
        self.E, self.H, self.T, self.R = E, H, T, R
        self.ET, self.HT, self.CT, self.RT = E // 128, H // 128, T // 128, R // 128
        self.scale = 1.0 / np.sqrt(E)


def _blocks(total, bs=512):
    return [(o, min(bs, total - o)) for o in range(0, total, bs)]


def build_program(cfg: Cfg, reps: int = 1):
    """Build the SPMD Bass program (one core's view).

    reps>1 wraps the body in unrolled copies — used only for timing
    (amortizes the ~80ms axon dispatch round-trip over reps executions).
    """
    E, H, T, R = cfg.E, cfg.H, cfg.T, cfg.R
    ET, HT, CT, RT = cfg.ET, cfg.HT, cfg.CT, cfg.RT

    nc = bacc.Bacc("TRN2", target_bir_lowering=False, debug=False,
                   num_devices=N_CORES)

    # ---- DRAM I/O ----
    xT8_in = nc.dram_tensor("xT8_in", [ET, 128, T], F8, kind="ExternalInput")
    xo8_in = nc.dram_tensor("xo8_in", [ET, 128, R], F8, kind="ExternalInput")
    x_b = nc.dram_tensor("x_b", [T, E], F32, kind="ExternalInput")
    x_own = nc.dram_tensor("x_own", [R, E], F32, kind="ExternalInput")
    mask_d = nc.dram_tensor("mask_d", [RT, 128, 256], BF16, kind="ExternalInput")
    wqt = nc.dram_tensor("wqt", [ET, 128, ET, 128], F8, kind="ExternalInput")
    wkt = nc.dram_tensor("wkt", [ET, 128, ET, 128], F8, kind="ExternalInput")
    wv = nc.dram_tensor("wv", [ET, 128, E], F8, kind="ExternalInput")
    bq = nc.dram_tensor("bq", [E], F32, kind="ExternalInput")
    bk = nc.dram_tensor("bk", [E], F32, kind="ExternalInput")
    bv = nc.dram_tensor("bv", [E], F32, kind="ExternalInput")
    w1t = nc.dram_tensor("w1t", [HT, 128, ET, 128], F8, kind="ExternalInput")
    w2at = nc.dram_tensor("w2at", [HT, 128, HT, 128], F8, kind="ExternalInput")
    w2bt = nc.dram_tensor("w2bt", [HT, 128, HT, 128], F8, kind="ExternalInput")
    w3m = nc.dram_tensor("w3m", [HT, 128, E], F8, kind="ExternalInput")
    b1 = nc.dram_tensor("b1", [H], F32, kind="ExternalInput")
    b2a = nc.dram_tensor("b2a", [H], F32, kind="ExternalInput")
    b2b = nc.dram_tensor("b2b", [H], F32, kind="ExternalInput")
    b3 = nc.dram_tensor("b3", [E], F32, kind="ExternalInput")
    scales = nc.dram_tensor("scales", [8], F32, kind="ExternalInput")
    ident_in = nc.dram_tensor("ident_in", [128, 128], BF16, kind="ExternalInput")
    nck = nc.dram_tensor("nck", [E], F32, kind="ExternalInput")
    ncq = nc.dram_tensor("ncq", [E], F32, kind="ExternalInput")
    ncv = nc.dram_tensor("ncv", [E], F32, kind="ExternalInput")
    nc1 = nc.dram_tensor("nc1", [H], F32, kind="ExternalInput")
    stat_ri = nc.dram_tensor("stat_ri", [T], F32, kind="Internal")
    stat_r2 = nc.dram_tensor("stat_r2", [T], F32, kind="Internal")
    stat_rio = nc.dram_tensor("stat_rio", [R], F32, kind="Internal")
    stat_r2o = nc.dram_tensor("stat_r2o", [R], F32, kind="Internal")
    stat_ri2 = nc.dram_tensor("stat_ri2", [R], F32, kind="Internal")
    stat_r22 = nc.dram_tensor("stat_r22", [R], F32, kind="Internal")
    out = nc.dram_tensor("out", [R, E], F32, kind="ExternalOutput")

    d = locals()
    with tile.TileContext(nc) as tc:
        for _ in range(reps):
            _emit(tc, cfg, d)
    nc.compile()
    return nc


def _emit(tc, cfg, d):
    nc = tc.nc
    E, H, T, R = cfg.E, cfg.H, cfg.T, cfg.R
    ET, HT, CT, RT = cfg.ET, cfg.HT, cfg.CT, cfg.RT
    c32 = float(cfg.scale)
    KP_E = ET // 2   # DoubleRow k-pair count for E contraction
    KP_H = HT // 2

    import contextlib
    ctx = contextlib.ExitStack()
    with ctx:
        consts = ctx.enter_context(tc.tile_pool(name="consts", bufs=1))
        mm_ps = ctx.enter_context(tc.tile_pool(name="mm_ps", bufs=6, space="PSUM"))
        tr_ps = ctx.enter_context(tc.tile_pool(name="tr_ps", bufs=2, space="PSUM"))
        stp = ctx.enter_context(tc.tile_pool(name="ln_stats", bufs=4))
        fxp = ctx.enter_context(tc.tile_pool(name="fixup", bufs=2))

        eps_t = consts.tile([128, 1], F32)
        nc.vector.memset(eps_t[:], EPS)
        ident = consts.tile([128, 128], BF16)
        nc.sync.dma_start(out=ident[:], in_=d["ident_in"].ap())
        neg1 = consts.tile([128, 1], F32)
        nc.vector.memset(neg1[:], -1.0)

        def bcast(name, dr, width=None):
            w = width or dr.shape[0]
            t = consts.tile([128, w], F32, tag=name)
            src = dr.ap()
            src_b = bass.AP(tensor=src.tensor, offset=src.offset,
                            ap=[[0, 128]] + list(src.ap))
            nc.sync.dma_start(out=t[:], in_=src_b)
            return t

        def cols(name, dr, nt):
            t = consts.tile([128, nt], F32, tag=name)
            nc.sync.dma_start(out=t[:], in_=dr.ap().rearrange("(t p) -> p t", p=128))
            return t

        sc = bcast("sc", d["scales"], width=8)
        SQ, SK, SV = sc[:, 0:1], sc[:, 1:2], sc[:, 2:3]
        S1, S2A, S2B, S3 = sc[:, 3:4], sc[:, 4:5], sc[:, 5:6], sc[:, 6:7]
        bq_c = cols("bq", d["bq"], ET)
        bk_c = cols("bk", d["bk"], ET)
        bv_bc = bcast("bv", d["bv"])
        b3_bc = bcast("b3", d["b3"])
        b1_c = cols("b1", d["b1"], HT)
        b2a_c = cols("b2a", d["b2a"], HT)
        b2b_c = cols("b2b", d["b2b"], HT)
        nck_c = cols("nck", d["nck"], ET)
        ncq_c = cols("ncq", d["ncq"], ET)
        ncv_bc = bcast("ncv", d["ncv"])
        nc1_c = cols("nc1", d["nc1"], HT)

        def tile_stats(src_ap, ri_slot, r2_slot):
            """Token-major LN stats of one [128, E] tile.

            ri = 1/sqrt(var+eps), r2 = mu*ri — the two per-token factors the
            folded-LN fixup needs (LN itself never materializes on-chip)."""
            scr = stp.tile([128, E], BF16, tag="scr")
            s1 = stp.tile([128, 1], F32, tag="s1")
            nc.scalar.activation(out=scr[:], in_=src_ap, func=AF.Copy, bias=0.0,
                                 scale=1.0, accum_out=s1[:])
            s2 = stp.tile([128, 1], F32, tag="s2")
            nc.scalar.activation(out=scr[:], in_=src_ap, func=AF.Square,
                                 accum_out=s2[:])
            mu = stp.tile([128, 1], F32, tag="mu_s")
            nc.scalar.mul(out=mu[:], in_=s1[:], mul=1.0 / E)
            mu2 = stp.tile([128, 1], F32, tag="mu2")
            nc.vector.tensor_mul(out=mu2[:], in0=mu[:], in1=mu[:])
            var = stp.tile([128, 1], F32, tag="var")
            nc.vector.scalar_tensor_tensor(out=var[:], in0=s2[:], scalar=1.0 / E,
                                           in1=mu2[:], op0=ALU.mult,
                                           op1=ALU.subtract)
            sd = stp.tile([128, 1], F32, tag="sd")
            nc.scalar.activation(out=sd[:], in_=var[:], func=AF.Sqrt,
                                 bias=eps_t[:], scale=1.0)
            nc.vector.reciprocal(out=ri_slot, in_=sd[:])
            nc.vector.tensor_mul(out=r2_slot, in0=mu[:], in1=ri_slot)

        x2_pool = ctx.enter_context(tc.tile_pool(name="x2", bufs=1))
        x2 = x2_pool.tile([128, RT, E], F32)  # residual stream (own rows)
        h2T_pool = ctx.enter_context(tc.tile_pool(name="h2T_pool", bufs=1))
        h2T = h2T_pool.tile([128, ET, R], F8, tag="h2T")
        ri2_b = h2T_pool.tile([128, R], F32, tag="ri2_b")
        r2_2b = h2T_pool.tile([128, R], F32, tag="r2_2b")

        # ================= attention block =================
        with tc.tile_pool(name="attn_big", bufs=1) as abig:
            xT8 = abig.tile([128, ET, T], F8, tag="xT8")
            xo8 = abig.tile([128, ET, R], F8, tag="xo8")

            qkvp = tc.tile_pool(name="qkvp", bufs=1)
            qkv_pool = qkvp.__enter__()
            statp = tc.tile_pool(name="statp", bufs=1)
            stat_pool = statp.__enter__()

            # x arrives pre-quantized to fp8 from the host (layout prep):
            # 3 MB across both HWDGE queues unblocks the projections in ~5 us
            for kt in range(ET):
                eng = (nc.sync, nc.scalar, nc.gpsimd)[kt % 3]
                eng.dma_start(out=xT8[:, kt, :], in_=d["xT8_in"].ap()[kt])
            for kt in range(ET):
                eng = (nc.scalar, nc.sync)[kt % 2]
                eng.dma_start(out=xo8[:, kt, :], in_=d["xo8_in"].ap()[kt])

            rio_all = stat_pool.tile([128, RT], F32, tag="rio_all")
            r2o_all = stat_pool.tile([128, RT], F32, tag="r2o_all")
            ri_all = stat_pool.tile([128, CT], F32, tag="ri_all")
            r2_all = stat_pool.tile([128, CT], F32, tag="r2_all")

            def brow(name, dr, w):
                t = stat_pool.tile([128, w], F32, tag=name, name="brow_t")
                src = dr.ap()
                src_b = bass.AP(tensor=src.tensor, offset=src.offset,
                                ap=[[0, 128]] + list(src.ap))
                nc.sync.dma_start(out=t[:], in_=src_b)
                return t

            qT8 = qkv_pool.tile([128, ET, T], F8, tag="qT8")
            kT8o = qkv_pool.tile([128, ET, R], F8, tag="kT8o")
            vtm = qkv_pool.tile([128, CT, E], F8, tag="vtm")  # token-major v

            def fixup_fm(ps, dst, ri_row, r2_row, ncol, bcol, S, func):
                """Feature-major eviction with folded LN:
                dst = func(S*ps*ri − c_m*r2 + b_m)   (exact LN algebra)."""
                o1 = fxp.tile([128, 512], F32, tag="fx1", name="fx_t")
                nc.vector.scalar_tensor_tensor(out=o1[:], in0=ps, scalar=S,
                                               in1=ri_row, op0=ALU.mult,
                                               op1=ALU.mult)
                o2 = fxp.tile([128, 512], F32, tag="fx2", name="fx_t")
                nc.vector.scalar_tensor_tensor(out=o2[:], in0=r2_row,
                                               scalar=ncol, in1=o1[:],
                                               op0=ALU.mult, op1=ALU.add)
                nc.scalar.activation(out=dst, in_=o2[:], func=func,
                                     bias=bcol, scale=1.0)

            # ---- v (full ctx, token-major), LN stats pipelined per tile ----
            with tc.tile_pool(name="wv_pool", bufs=1) as wvp, \
                 tc.tile_pool(name="ln1w", bufs=3) as lnw:
                wv_sb = wvp.tile([128, ET, E], F8)
                for kt in range(ET):
                    nc.scalar.dma_start(out=wv_sb[:, kt, :], in_=d["wv"].ap()[kt])
                def ctx_stats(st):
                    xt = lnw.tile([128, E], F32, tag="xt", bufs=2, name="xt_t")
                    nc.gpsimd.dma_start(out=xt[:],
                                        in_=d["x_b"].ap()[st * 128:(st + 1) * 128, :])
                    tile_stats(xt[:], ri_all[:, st:st + 1], r2_all[:, st:st + 1])
                def own_stats(it):
                    xt = lnw.tile([128, E], F32, tag="xt", bufs=2, name="xt_t")
                    nc.gpsimd.dma_start(out=xt[:],
                                        in_=d["x_own"].ap()[it * 128:(it + 1) * 128, :])
                    tile_stats(xt[:], rio_all[:, it:it + 1], r2o_all[:, it:it + 1])
                for tt in range(CT):
                    # ctx stats run two tiles ahead of the v evictions; own
                    # stats (k-path) fill the second half so the k roundtrip
                    # is ready the moment the q loop drains
                    if tt < CT // 2:
                        ctx_stats(2 * tt)
                        ctx_stats(2 * tt + 1)
                    else:
                        own_stats(tt - CT // 2)
                    pss = [mm_ps.tile([128, 512], F32, tag="mm", name="mm_ps_t")
                           for _ in range(2)]
                    for kp in range(KP_E):
                        for bi, eo in enumerate((0, 512)):
                            nc.tensor.matmul(
                                pss[bi][:, :],
                                xT8[:, 2 * kp:2 * kp + 2, tt * 128:(tt + 1) * 128],
                                wv_sb[:, 2 * kp:2 * kp + 2, eo:eo + 512],
                                start=(kp == 0), stop=(kp == KP_E - 1),
                                perf_mode=DR)
                    for bi, eo in enumerate((0, 512)):
                        o1 = fxp.tile([128, 512], F32, tag="fx1", name="fx_t")
                        nc.vector.tensor_scalar(out=o1[:], in0=pss[bi][:, :],
                                                scalar1=SV,
                                                scalar2=ri_all[:, tt:tt + 1],
                                                op0=ALU.mult, op1=ALU.mult)
                        o2 = fxp.tile([128, 512], F32, tag="fx2", name="fx_t")
                        nc.vector.scalar_tensor_tensor(
                            out=o2[:], in0=ncv_bc[:, eo:eo + 512],
                            scalar=r2_all[:, tt:tt + 1], in1=o1[:],
                            op0=ALU.mult, op1=ALU.add)
                        nc.vector.tensor_tensor(out=vtm[:, tt, eo:eo + 512],
                                                in0=o2[:],
                                                in1=bv_bc[:, eo:eo + 512],
                                                op=ALU.add)
                nc.sync.dma_start(
                    out=d["stat_ri"].ap().rearrange("(t p) -> p t", p=128),
                    in_=ri_all[:])
                nc.sync.dma_start(
                    out=d["stat_r2"].ap().rearrange("(t p) -> p t", p=128),
                    in_=r2_all[:])
                ri_b = brow("ri_b", d["stat_ri"], T)
                r2_b = brow("r2_b", d["stat_r2"], T)

                # own-rows roundtrip (stats computed during the v loop)
                nc.sync.dma_start(
                    out=d["stat_rio"].ap().rearrange("(t p) -> p t", p=128),
                    in_=rio_all[:])
                nc.sync.dma_start(
                    out=d["stat_r2o"].ap().rearrange("(t p) -> p t", p=128),
                    in_=r2o_all[:])
                ri_bo = brow("ri_bo", d["stat_rio"], R)
                r2_bo = brow("r2_bo", d["stat_r2o"], R)

                # ---- q (full ctx), feature-major ----
                with tc.tile_pool(name="wq_s", bufs=2) as wqs:
                    for mt in range(ET):
                        wq_mt = wqs.tile([128, ET, 128], F8, tag="wq_mt", bufs=3)
                        nc.scalar.dma_start(out=wq_mt[:], in_=d["wqt"].ap()[mt])
                        pss = [mm_ps.tile([128, 512], F32, tag="mm",
                                          name="mm_ps_t") for _ in range(4)]
                        for kp in range(KP_E):
                            for bi in range(4):
                                nc.tensor.matmul(
                                    pss[bi][:, :], wq_mt[:, 2 * kp:2 * kp + 2, :],
                                    xT8[:, 2 * kp:2 * kp + 2,
                                        bi * 512:(bi + 1) * 512],
                                    start=(kp == 0), stop=(kp == KP_E - 1),
                                    perf_mode=DR)
                        for bi in range(4):
                            fixup_fm(pss[bi][:, :],
                                     qT8[:, mt, bi * 512:(bi + 1) * 512],
                                     ri_b[:, bi * 512:(bi + 1) * 512],
                                     r2_b[:, bi * 512:(bi + 1) * 512],
                                     ncq_c[:, mt:mt + 1], bq_c[:, mt:mt + 1],
                                     SQ, AF.Identity)

                # ---- k (own rows), feature-major ----
                with tc.tile_pool(name="wk_s", bufs=2) as wks:
                    for mt in range(ET):
                        wk_mt = wks.tile([128, ET, 128], F8, tag="wk_mt", bufs=3)
                        nc.scalar.dma_start(out=wk_mt[:], in_=d["wkt"].ap()[mt])
                        pss = [mm_ps.tile([128, 512], F32, tag="mm",
                                          name="mm_ps_t") for _ in range(2)]
                        for kp in range(KP_E):
                            for bi, ro in enumerate((0, 512)):
                                nc.tensor.matmul(
                                    pss[bi][:, :], wk_mt[:, 2 * kp:2 * kp + 2, :],
                                    xo8[:, 2 * kp:2 * kp + 2, ro:ro + 512],
                                    start=(kp == 0), stop=(kp == KP_E - 1),
                                    perf_mode=DR)
                        for bi, ro in enumerate((0, 512)):
                            fixup_fm(pss[bi][:, :], kT8o[:, mt, ro:ro + 512],
                                     ri_bo[:, ro:ro + 512], r2_bo[:, ro:ro + 512],
                                     nck_c[:, mt:mt + 1], bk_c[:, mt:mt + 1],
                                     SK, AF.Identity)

            statp.__exit__(None, None, None)

            # ---- attention rows (own i-tiles), software-pipelined ----
            # Core r owns batch i-tiles {2*it + r}; padded causal extent
            # ext(it) = 2*(it+1) j-tiles is core-independent.  Only the two
            # diagonal j-tiles need masking (mask_d input); earlier blocks are
            # fully open and exp() reads the score PSUM directly.  No
            # max-subtraction: |sim/32| <= ~11 keeps exp in fp32/bf16 range.
            # Scores of i-tile it+1 are emitted before AV of it so the PE works
            # while the pT XBAR-DMA transposes of it are in flight.
            with tc.tile_pool(name="at_p", bufs=2) as pp, \
                 tc.tile_pool(name="at_misc", bufs=3) as msc, \
                 tc.tile_pool(name="at_md", bufs=2) as mdp, \
                 tc.tile_pool(name="ln2", bufs=2) as l2p:
                ri2_all = l2p.tile([128, RT], F32, tag="ri2_all", bufs=1)
                r22_all = l2p.tile([128, RT], F32, tag="r22_all", bufs=1)
                s1_all = l2p.tile([128, RT], F32, tag="s1_all", bufs=1)
                s2_all = l2p.tile([128, RT], F32, tag="s2_all", bufs=1)
                def scores_phase(it):
                    ext = 2 * (it + 1)
                    ncols = ext * 128
                    blks = _blocks(ncols)
                    nblk = len(blks)
                    md = mdp.tile([128, 256], BF16, tag="md", name="at_t")
                    nc.sync.dma_start(out=md[:], in_=d["mask_d"].ap()[it])
                    pss = [mm_ps.tile([128, 512], F32, tag="mm", name="mm_ps_t")
                           for _ in range(nblk)]
                    for kp in range(KP_E):
                        for bi, (jo, jn) in enumerate(blks):
                            nc.tensor.matmul(
                                pss[bi][:, :jn],
                                kT8o[:, 2 * kp:2 * kp + 2, it * 128:(it + 1) * 128],
                                qT8[:, 2 * kp:2 * kp + 2, jo:jo + jn],
                                start=(kp == 0), stop=(kp == KP_E - 1),
                                perf_mode=DR)
                    pbf = pp.tile([128, T], BF16, tag="pbf", name="at_t")
                    lacc = msc.tile([128, 8], F32, tag="lacc", name="at_t")
                    ns = 0
                    for bi, (jo, jn) in enumerate(blks):
                        last = (bi == nblk - 1)
                        jn_open = jn - 256 if last else jn
                        if jn_open > 0:
                            nc.scalar.activation(
                                out=pbf[:, jo:jo + jn_open],
                                in_=pss[bi][:, :jn_open], func=AF.Exp,
                                scale=c32, bias=neg1[:],
                                accum_out=lacc[:, ns:ns + 1])
                            ns += 1
                        if last:
                            simd = msc.tile([128, 256], F32, tag="simd",
                                            name="at_t")
                            nc.vector.tensor_tensor(out=simd[:],
                                                    in0=pss[bi][:, jn_open:jn],
                                                    in1=md[:], op=ALU.add)
                            nc.scalar.activation(
                                out=pbf[:, ncols - 256:ncols], in_=simd[:],
                                func=AF.Exp, scale=c32, bias=neg1[:],
                                accum_out=lacc[:, ns:ns + 1])
                            ns += 1
                    lrow = msc.tile([128, 1], F32, tag="lrow", name="at_t")
                    nc.vector.tensor_reduce(out=lrow[:], in_=lacc[:, :ns],
                                            axis=AX.X, op=ALU.add)
                    linv = msc.tile([128, 1], F32, tag="linv", name="at_t")
                    nc.vector.reciprocal(out=linv[:], in_=lrow[:])
                    return ext, pbf, linv

                def transpose_phase(ext, pbf):
                    pT = pp.tile([128, T], F8E5, tag="pT", name="at_t")
                    for jt in range(ext):
                        tp = tr_ps.tile([128, 128], BF16, tag="tr", name="tr_t")
                        nc.tensor.transpose(tp[:], pbf[:, jt * 128:(jt + 1) * 128],
                                            ident[:])
                        nc.vector.tensor_copy(out=pT[:, jt * 128:(jt + 1) * 128],
                                              in_=tp[:])
                    return pT

                def av_phase(it, ext, pT, linv):
                    xo = msc.tile([128, E], F32, tag="xo", bufs=2, name="at_t")
                    nc.gpsimd.dma_start(out=xo[:],
                                        in_=d["x_own"].ap()[it * 128:(it + 1) * 128, :])
                    pse = [mm_ps.tile([128, 512], F32, tag="mm", name="mm_ps_t")
                           for _ in range(2)]
                    for jp in range(ext // 2):
                        pT_pair = pT[:, jp * 256:(jp + 1) * 256].rearrange(
                            "p (two c) -> p two c", two=2)
                        for bi, eo in enumerate((0, 512)):
                            nc.tensor.matmul(
                                pse[bi][:, :], pT_pair,
                                vtm[:, 2 * jp:2 * jp + 2, eo:eo + 512],
                                start=(jp == 0), stop=(jp == ext // 2 - 1),
                                perf_mode=DR)
                    for bi, eo in enumerate((0, 512)):
                        nc.vector.scalar_tensor_tensor(
                            out=x2[:, it, eo:eo + 512], in0=pse[bi][:, :],
                            scalar=linv[:], in1=xo[:, eo:eo + 512],
                            op0=ALU.mult, op1=ALU.add)
                    # LN2 for this row-tile, fused so DVE/ACT/XBAR do it while
                    # the PE continues with the next i-tile's scores
                    x2bf = l2p.tile([128, E], BF16, tag="x2bf", name="l2_t")
                    nc.vector.tensor_copy(out=x2bf[:], in_=x2[:, it, :])
                    h2bf = l2p.tile([128, E], BF16, tag="h2bf", name="l2_t")
                    for et in range(ET):
                        eng = nc.sync if et % 2 == 0 else nc.scalar
                        eng.dma_start(out=h2bf[:, et * 128:(et + 1) * 128],
                                      in_=x2bf[:, et * 128:(et + 1) * 128],
                                      transpose=True)
                    nc.vector.tensor_copy(
                        out=h2T[:, :, it * 128:(it + 1) * 128],
                        in_=h2bf[:].rearrange("p (et c) -> p et c", c=128))
                    scr2 = stp.tile([128, E], BF16, tag="scr2", name="l2_t")
                    nc.scalar.activation(out=scr2[:], in_=x2[:, it, :],
                                         func=AF.Copy, bias=0.0, scale=1.0,
                                         accum_out=s1_all[:, it:it + 1])
                    scr3 = stp.tile([128, E], BF16, tag="scr3", name="l2_t")
                    nc.scalar.activation(out=scr3[:], in_=x2[:, it, :],
                                         func=AF.Square,
                                         accum_out=s2_all[:, it:it + 1])
                    nc.vector.tensor_tensor(out=x2[:, it, :], in0=x2[:, it, :],
                                            in1=b3_bc[:], op=ALU.add)

                pend = None
                for it in range(RT):
                    ext, pbf, linv = scores_phase(it)
                    if pend is not None:
                        av_phase(pend[0], *pend[1])
                    pT = transpose_phase(ext, pbf)
                    pend = (it, (ext, pT, linv))
                av_phase(pend[0], *pend[1])
                mu8 = stp.tile([128, RT], F32, tag="mu8", name="l2_t")
                nc.scalar.mul(out=mu8[:], in_=s1_all[:], mul=1.0 / E)
                mu28 = stp.tile([128, RT], F32, tag="mu28", name="l2_t")
                nc.vector.tensor_mul(out=mu28[:], in0=mu8[:], in1=mu8[:])
                var8 = stp.tile([128, RT], F32, tag="var8", name="l2_t")
                nc.vector.scalar_tensor_tensor(out=var8[:], in0=s2_all[:],
                                               scalar=1.0 / E, in1=mu28[:],
                                               op0=ALU.mult, op1=ALU.subtract)
                sd8 = stp.tile([128, RT], F32, tag="sd8", name="l2_t")
                nc.scalar.activation(out=sd8[:], in_=var8[:], func=AF.Sqrt,
                                     bias=eps_t[:], scale=1.0)
                nc.vector.reciprocal(out=ri2_all[:], in_=sd8[:])
                nc.vector.tensor_mul(out=r22_all[:], in0=mu8[:], in1=ri2_all[:])
                nc.sync.dma_start(
                    out=d["stat_ri2"].ap().rearrange("(t p) -> p t", p=128),
                    in_=ri2_all[:])
                nc.sync.dma_start(
                    out=d["stat_r22"].ap().rearrange("(t p) -> p t", p=128),
                    in_=r22_all[:])
                srcap = d["stat_ri2"].ap()
                nc.sync.dma_start(out=ri2_b[:], in_=bass.AP(
                    tensor=srcap.tensor, offset=srcap.offset,
                    ap=[[0, 128]] + list(srcap.ap)))
                srcap = d["stat_r22"].ap()
                nc.sync.dma_start(out=r2_2b[:], in_=bass.AP(
                    tensor=srcap.tensor, offset=srcap.offset,
                    ap=[[0, 128]] + list(srcap.ap)))
            qkvp.__exit__(None, None, None)

        # ================= MLP block =================
        with tc.tile_pool(name="gx", bufs=1) as gxp, \
             tc.tile_pool(name="mlp_ws", bufs=1) as ws:
            g1T = gxp.tile([128, HT, R], F8, tag="gx")
            for mt in range(HT):
                w1_mt = ws.tile([128, ET, 128], F8, tag="w1_mt", bufs=3)
                nc.scalar.dma_start(out=w1_mt[:], in_=d["w1t"].ap()[mt])
                pss = [mm_ps.tile([128, 512], F32, tag="mm", name="mm_ps_t")
                       for _ in range(2)]
                for kp in range(KP_E):
                    for bi, ro in enumerate((0, 512)):
                        nc.tensor.matmul(
                            pss[bi][:, :], w1_mt[:, 2 * kp:2 * kp + 2, :],
                            h2T[:, 2 * kp:2 * kp + 2, ro:ro + 512],
                            start=(kp == 0), stop=(kp == KP_E - 1), perf_mode=DR)
                for bi, ro in enumerate((0, 512)):
                    fixup_fm(pss[bi][:, :], g1T[:, mt, ro:ro + 512],
                             ri2_b[:, ro:ro + 512], r2_2b[:, ro:ro + 512],
                             nc1_c[:, mt:mt + 1], b1_c[:, mt:mt + 1],
                             S1, AF.Gelu)

            with tc.tile_pool(name="g2", bufs=1) as g2p:
                g2T = g2p.tile([128, HT, R], F8, tag="g2")
                for mt in range(HT):
                    w2_mt = ws.tile([128, HT, 128], F8, tag="w2a_mt", bufs=3)
                    nc.scalar.dma_start(out=w2_mt[:], in_=d["w2at"].ap()[mt])
                    pss = [mm_ps.tile([128, 512], F32, tag="mm", name="mm_ps_t")
                           for _ in range(2)]
                    for kp in range(KP_H):
                        for bi, ro in enumerate((0, 512)):
                            nc.tensor.matmul(
                                pss[bi][:, :], w2_mt[:, 2 * kp:2 * kp + 2, :],
                                g1T[:, 2 * kp:2 * kp + 2, ro:ro + 512],
                                start=(kp == 0), stop=(kp == KP_H - 1),
                                perf_mode=DR)
                    for bi, ro in enumerate((0, 512)):
                        nc.scalar.activation(out=g2T[:, mt, ro:ro + 512],
                                             in_=pss[bi][:, :], func=AF.Gelu,
                                             bias=b2a_c[:, mt:mt + 1], scale=S2A)

                g3T = gxp.tile([128, HT, R], F8, tag="gx")
                for mt in range(HT):
                    w2_mt = ws.tile([128, HT, 128], F8, tag="w2b_mt", bufs=3)
                    nc.scalar.dma_start(out=w2_mt[:], in_=d["w2bt"].ap()[mt])
                    pss = [mm_ps.tile([128, 512], F32, tag="mm", name="mm_ps_t")
                           for _ in range(2)]
                    for kp in range(KP_H):
                        for bi, ro in enumerate((0, 512)):
                            nc.tensor.matmul(
                                pss[bi][:, :], w2_mt[:, 2 * kp:2 * kp + 2, :],
                                g2T[:, 2 * kp:2 * kp + 2, ro:ro + 512],
                                start=(kp == 0), stop=(kp == KP_H - 1),
                                perf_mode=DR)
                    for bi, ro in enumerate((0, 512)):
                        nc.scalar.activation(out=g3T[:, mt, ro:ro + 512],
                                             in_=pss[bi][:, :], func=AF.Gelu,
                                             bias=b2b_c[:, mt:mt + 1], scale=S2B)

            # ---- f = g3 @ W3 (+b3 already in x2); out = x2 + f ----
            with tc.tile_pool(name="w3p", bufs=1) as w3p, \
                 tc.tile_pool(name="outp", bufs=2) as op:
                w3_sb = w3p.tile([128, HT, E], F8)
                for kt in range(HT):
                    nc.scalar.dma_start(out=w3_sb[:, kt, :], in_=d["w3m"].ap()[kt])
                for tt in range(RT):
                    pse = [mm_ps.tile([128, 512], F32, tag="mm", name="mm_ps_t")
                           for _ in range(2)]
                    for kp in range(KP_H):
                        for bi, eo in enumerate((0, 512)):
                            nc.tensor.matmul(
                                pse[bi][:, :],
                                g3T[:, 2 * kp:2 * kp + 2, tt * 128:(tt + 1) * 128],
                                w3_sb[:, 2 * kp:2 * kp + 2, eo:eo + 512],
                                start=(kp == 0), stop=(kp == KP_H - 1),
                                perf_mode=DR)
                    for bi, eo in enumerate((0, 512)):
                        ot = op.tile([128, 512], F32, tag="ot")
                        nc.vector.scalar_tensor_tensor(
                            out=ot[:], in0=pse[bi][:, :], scalar=S3,
                            in1=x2[:, tt, eo:eo + 512], op0=ALU.mult, op1=ALU.add)
                        eng = nc.sync if bi == 0 else nc.scalar
                        eng.dma_start(
                            out=d["out"].ap()[tt * 128:(tt + 1) * 128, eo:eo + 512],
                            in_=ot[:])


# ---------------- host side ----------------

NPF8 = ml_dtypes.float8_e4m3  # TRN FP8_EXP4 semantics (bias 7, max 240)


def _f8_scale(w):
    """Power-of-two scale mapping amax into (64, 128] — fp8's sweet spot."""
    amax = float(np.abs(w).max())
    if amax == 0.0:
        return 1.0
    return float(2.0 ** np.floor(np.log2(128.0 / amax)))


def _tile_lhs_f8(wq):
    """Quantized [K, M] -> [MT, 128, KT, 128] (per-m-tile lhsT blocks)."""
    K, M = wq.shape
    t = wq.reshape(K // 128, 128, M // 128, 128).transpose(2, 1, 0, 3)
    return np.ascontiguousarray(t)


def _rows_f8(wq):
    """Quantized [K, N] -> [KT, 128, N] (k-partitioned moving layout)."""
    K, N = wq.shape
    return np.ascontiguousarray(wq.reshape(K // 128, 128, N))


def own_rows(cfg: Cfg, r):
    """Row indices (within the batch) owned by core half r: i-tiles {2j+r}."""
    tiles = [2 * it + r for it in range(cfg.RT)]
    return np.concatenate([np.arange(t * 128, (t + 1) * 128) for t in tiles])


def prepare_core_inputs(inputs, cfg: Cfg, b, r):
    E, T, R, ET, RT = cfg.E, cfg.T, cfg.R, cfg.ET, cfg.RT
    x = np.asarray(inputs["x"])
    rows = own_rows(cfg, r)
    xb = np.ascontiguousarray(x[b]).astype(np.float32)
    x_own = np.ascontiguousarray(xb[rows])
    im = {
        "x_b": xb,
        "x_own": x_own,
        "xT8_in": np.ascontiguousarray(xb.T).reshape(ET, 128, T).astype(NPF8),
        "xo8_in": np.ascontiguousarray(x_own.T).reshape(ET, 128, R).astype(NPF8),
        "ident_in": np.eye(128, dtype=ml_dtypes.bfloat16),
    }
    md = np.empty((RT, 128, 256), np.float32)
    for it in range(RT):
        i_glob = rows[it * 128:(it + 1) * 128]
        j_glob = 256 * it + np.arange(256)
        md[it] = np.where(j_glob[None, :] <= i_glob[:, None], 0.0, -1e30)
    im["mask_d"] = md.astype(ml_dtypes.bfloat16)
    return im


def prepare_shared_weights(inputs, cfg: Cfg):
    """Quantize/tile/scale weights; fold the LN affines into the downstream
    matmuls:  (n*w + b) @ W + c  ==  n @ (diag(w) W) + (b @ W + c).
    The folded-LN colsum corrections (nck/ncq/ncv/nc1) are computed from the
    QUANTIZED weights so the on-device mean subtraction is exact."""
    ln1_w, ln1_b = np.asarray(inputs["ln1_w"]), np.asarray(inputs["ln1_b"])
    ln2_w, ln2_b = np.asarray(inputs["ln2_w"]), np.asarray(inputs["ln2_b"])
    Wq, Wk, Wv = (np.asarray(inputs[k]) for k in ("Wq", "Wk", "Wv"))
    W1 = np.asarray(inputs["W1"])
    wq_e = ln1_w[:, None] * Wq
    wk_e = ln1_w[:, None] * Wk
    wv_e = ln1_w[:, None] * Wv
    bq_e = ln1_b @ Wq + np.asarray(inputs["bq"])
    bk_e = ln1_b @ Wk + np.asarray(inputs["bk"])
    bv_e = ln1_b @ Wv + np.asarray(inputs["bv"])
    w1_e = ln2_w[:, None] * W1
    b1_e = ln2_b @ W1 + np.asarray(inputs["b1"])
    W2a, W2b, W3 = (np.asarray(inputs[k]) for k in ("W2a", "W2b", "W3"))

    def quant(w):
        s = _f8_scale(w)
        return (w * s).astype(NPF8), s

    wq_q, s_q = quant(wq_e)
    wk_q, s_k = quant(wk_e)
    wv_q, s_v = quant(wv_e)
    w1_q, s_1 = quant(w1_e)
    w2a_q, s_2a = quant(W2a)
    w2b_q, s_2b = quant(W2b)
    w3_q, s_3 = quant(W3)

    def ncsum(wq_, s):
        return (-wq_.astype(np.float32).sum(axis=0) / s).astype(np.float32)

    return {
        "wqt": _tile_lhs_f8(wq_q),
        "wkt": _tile_lhs_f8(wk_q),
        "wv": _rows_f8(wv_q),
        "bq": bq_e.astype(np.float32), "bk": bk_e.astype(np.float32),
        "bv": bv_e.astype(np.float32),
        "w1t": _tile_lhs_f8(w1_q),
        "b1": b1_e.astype(np.float32),
        "w2at": _tile_lhs_f8(w2a_q),
        "w2bt": _tile_lhs_f8(w2b_q),
        "w3m": _rows_f8(w3_q),
        "b2a": np.asarray(inputs["b2a"]).astype(np.float32),
        "b2b": np.asarray(inputs["b2b"]).astype(np.float32),
        "b3": np.asarray(inputs["b3"]).astype(np.float32),
        "scales": np.array([1 / s_q, 1 / s_k, 1 / s_v, 1 / s_1,
                            1 / s_2a, 1 / s_2b, 1 / s_3, 0.0], np.float32),
        "nck": ncsum(wk_q, s_k),
        "ncq": ncsum(wq_q, s_q),
        "ncv": ncsum(wv_q, s_v),
        "nc1": ncsum(w1_q, s_1),
    }


_PROGRAM_CACHE = {}


def get_program(cfg: Cfg, reps: int = 1):
    key = (cfg.E, cfg.H, cfg.T, cfg.R, reps)
    if key not in _PROGRAM_CACHE:
        _PROGRAM_CACHE[key] = build_program(cfg, reps=reps)
    return _PROGRAM_CACHE[key]


def run(inputs, cfg: Cfg, trace=False):
    nc = get_program(cfg)
    shared = prepare_shared_weights(inputs, cfg)
    in_maps = []
    for c in range(N_CORES):
        b, r = c // 2, c % 2
        im = prepare_core_inputs(inputs, cfg, b, r)
        im.update(shared)
        in_maps.append(im)
    res = run_bass_kernel_spmd(nc, in_maps, core_ids=list(range(N_CORES)),
                               trace=trace)
    B = np.asarray(inputs["x"]).shape[0]
    outp = np.empty((B, cfg.T, cfg.E), np.float32)
    for c in range(N_CORES):
        b, r = c // 2, c % 2
        outp[b][own_rows(cfg, r)] = res.results[c]["out"]
    return outp, res


def _build_sharded_exec(nc, in_maps):
    """Mirror bass2jax.run_bass_via_pjrt but return a reusable timed runner."""
    import jax
    from jax.sharding import Mesh, PartitionSpec, NamedSharding
    from jax.experimental.shard_map import shard_map
    import concourse.mybir as mb
    from concourse import bass2jax

    bass2jax.install_neuronx_cc_hook()
    n_cores = len(in_maps)
    partition_name = (nc.partition_id_tensor.name
                      if nc.partition_id_tensor is not None else None)
    in_names, out_names, out_avals, zero_outs = [], [], [], []
    for alloc in nc.m.functions[0].allocations:
        if not isinstance(alloc, mb.MemoryLocationSet):
            continue
        name = alloc.memorylocations[0].name
        if alloc.kind == "ExternalInput":
            if name != partition_name:
                in_names.append(name)
        elif alloc.kind == "ExternalOutput":
            out_names.append(name)
            shape = tuple(alloc.tensor_shape)
            dtype = mb.dt.np(alloc.dtype)
            out_avals.append(jax.core.ShapedArray(shape, dtype))
            zero_outs.append(np.zeros(shape, dtype))
    n_params = len(in_names)
    n_outs = len(out_avals)
    all_names = in_names + out_names
    if partition_name is not None:
        all_names = all_names + [partition_name]

    def _call_once(params, zouts):
        operands = list(params) + list(zouts)
        if partition_name is not None:
            operands.append(bass2jax.partition_id_tensor())
        outs = bass2jax._bass_exec_p.bind(
            *operands,
            out_avals=tuple(out_avals),
            in_names=tuple(all_names),
            out_names=tuple(out_names),
            lowering_input_output_aliases=(),
            sim_require_finite=True,
            sim_require_nnan=True,
            nc=nc,
        )
        return tuple(outs)

    def make_body(chain):
        def _body(*args):
            params = args[:n_params]
            outs = args[n_params:]
            for _ in range(chain):
                outs = _call_once(params, outs)
            return tuple(outs)
        return _body

    devices = jax.devices()[:n_cores]
    mesh = Mesh(np.asarray(devices), ("core",))
    in_specs = (PartitionSpec("core"),) * (n_params + n_outs)
    out_specs = (PartitionSpec("core"),) * n_outs
    donate = tuple(range(n_params, n_params + n_outs))

    def make_sharded(chain):
        return jax.jit(
            shard_map(make_body(chain), mesh=mesh, in_specs=in_specs,
                      out_specs=out_specs, check_rep=False),
            donate_argnums=donate, keep_unused=True)

    sharded = make_sharded(1)

    sh = NamedSharding(mesh, PartitionSpec("core"))
    concat_in = [
        jax.device_put(
            np.concatenate([np.asarray(in_maps[c][nm]) for c in range(n_cores)],
                           axis=0), sh)
        for nm in in_names
    ]

    def make_zeros():
        return [jax.device_put(
            np.zeros((n_cores * z.shape[0], *z.shape[1:]), z.dtype), sh)
            for z in zero_outs]

    _jit_cache = {1: sharded}

    def runner(chain=1, nruns=1):
        if chain not in _jit_cache:
            _jit_cache[chain] = make_sharded(chain)
        fn = _jit_cache[chain]
        all_zs = [make_zeros() for _ in range(nruns)]
        for zs in all_zs:
            for z in zs:
                z.block_until_ready()
        t0 = time.perf_counter()
        outs_l = [fn(*concat_in, *zs) for zs in all_zs]
        for outs in outs_l:
            for o in outs:
                o.block_until_ready()
        return time.perf_counter() - t0, outs_l[-1]

    return runner, out_names


def _make_in_maps(inputs, cfg: Cfg):
    shared = prepare_shared_weights(inputs, cfg)
    in_maps = []
    for c in range(N_CORES):
        b, r = c // 2, c % 2
        im = prepare_core_inputs(inputs, cfg, b, r)
        im.update(shared)
        in_maps.append(im)
    return in_maps


def time_exec(inputs, cfg: Cfg, iters=8, reps=3):
    """Per-execution device time via a NEFF containing `reps` unrolled copies
    of the kernel body, differenced against reps=1 to cancel the ~80 ms axon
    dispatch round-trip.  Returns (per_exec_estimate, t1_list, tk_list)."""
    in_maps = _make_in_maps(inputs, cfg)
    r1, _ = _build_sharded_exec(get_program(cfg, reps=1), in_maps)
    rk, _ = _build_sharded_exec(get_program(cfg, reps=reps), in_maps)
    r1(); rk()  # warm both
    t1s, tks = [], []
    for _ in range(iters):
        t1, _ = r1()
        tk, _ = rk()
        t1s.append(t1)
        tks.append(tk)
    med = (np.median(tks) - np.median(t1s)) / (reps - 1)
    return med, t1s, tks


def kernel(**inputs) -> np.ndarray:
    cfg = Cfg(E=1024, H=4096, T=2048, R=1024)
    outp, _ = run(inputs, cfg)
    return outp
